# revision 1
# baseline (speedup 1.0000x reference)
"""GAT (2-layer, 3-head) forward on 8 Trainium2 NeuronCores.

Math: with LeakyReLU slope ALPHA=1.0 the edge score e_ij = s1_i + s2_j is
linear, and s1_i cancels inside the row softmax.  The masked softmax over
j therefore reduces to column weights w_j = exp(s2_j - C) restricted to
adj, giving

    h'_i = (sum_j adj_ij * w_j * h_j) / (sum_j adj_ij * w_j)

i.e. one adjacency matmul against G = [w*h | w].  Both GAT layers take
this form (the same adjacency masks both), so the whole network is two
A-matmuls plus small projections.

Sharding: rows of h' (nodes) across 8 cores; each core holds lhsT-layout
adjacency columns A^T[:, slab] and computes its 512-row slab.  The
G matrices are built slab-wise and AllGathered per head so the gathers
pipeline against the adjacency matmul.  s2 = x @ (W a2) is computed up
front from host-folded u vectors so the max-reduction collective hides
under the x@W phase.  Matmuls run in bf16 hi/lo pair precision (~17
mantissa bits), accumulating in fp32 PSUM.
"""
import sys

sys.path.insert(0, "/opt/trn_rl_repo")

import numpy as np
import ml_dtypes

import concourse.bass as bass
import concourse.bacc as bacc
import concourse.mybir as mybir
import concourse.bass_isa as bass_isa
import concourse.tile as tile
from concourse.bass_utils import run_bass_kernel_spmd

BF16 = ml_dtypes.bfloat16

N = 4096
F = 768
HID = 768
NH = 3
NCLS = 256
NCORES = 8
SLAB = N // NCORES          # 512 rows per core
NIT = SLAB // 128           # 4 i-tiles per core
NJT = N // 128              # 32 j-tiles
NFT = F // 128              # 6 f-tiles
GH = NH * HID               # 2304 scaled-feature columns
NCT = NH * NFT              # 18 feature col-tiles of G
G2C = NCLS + 1              # 257 = classes + s2' column (folded u2)
PAD2 = 264                  # G2 half padded to 32B rows
WCOLS = 16                  # w-column slab width (3 used + pad)

AF = mybir.ActivationFunctionType
ALU = mybir.AluOpType


def _enable_ldw_opt():
    # walrus defaults to --enable-ldw-opt=false; with it off every LDWEIGHTS
    # serializes against the previous matmul (~427ns vs ~213ns per 512-col
    # matmul).  Patch the arg builder so the stationary loads pipeline.
    import concourse.bass_utils as _bu
    if getattr(_bu, "_ldw_opt_patched", False):
        return
    _orig = _bu.get_walrus_args

    def _patched(*a, **k):
        args = _orig(*a, **k)
        return [x.replace("--enable-ldw-opt=false", "--enable-ldw-opt=true")
                for x in args]

    _bu.get_walrus_args = _patched
    _bu._ldw_opt_patched = True


def build():
    dt = mybir.dt
    _enable_ldw_opt()
    nc = bacc.Bacc(num_devices=NCORES)

    adjT_d = nc.dram_tensor("adjT", [N, SLAB], dt.bfloat16, kind="ExternalInput")
    xTh_d = nc.dram_tensor("xT_hi", [F, SLAB], dt.bfloat16, kind="ExternalInput")
    xTl_d = nc.dram_tensor("xT_lo", [F, SLAB], dt.bfloat16, kind="ExternalInput")
    U6_d = nc.dram_tensor("U6", [F, 8], dt.bfloat16, kind="ExternalInput")
    U3_d = nc.dram_tensor("U3", [F, 8], dt.bfloat16, kind="ExternalInput")
    Wh_d = nc.dram_tensor("W_hi", [NH, F, HID], dt.bfloat16, kind="ExternalInput")
    Wl_d = nc.dram_tensor("W_lo", [NH, F, HID], dt.bfloat16, kind="ExternalInput")
    Woh_d = nc.dram_tensor("Wo_hi", [GH, G2C], dt.bfloat16, kind="ExternalInput")
    Wol_d = nc.dram_tensor("Wo_lo", [GH, G2C], dt.bfloat16, kind="ExternalInput")
    out_d = nc.dram_tensor("out", [SLAB, NCLS], dt.float32, kind="ExternalOutput")

    # DRAM scratch + collective buffers
    gs = [nc.dram_tensor(f"gs{h}", [SLAB, 2 * HID], dt.bfloat16) for h in range(NH)]
    gf = [nc.dram_tensor(f"gf{h}", [N, 2 * HID], dt.bfloat16, addr_space="Shared")
          for h in range(NH)]
    gsw = nc.dram_tensor("gsw", [SLAB, WCOLS], dt.bfloat16)
    gfw = nc.dram_tensor("gfw", [N, WCOLS], dt.bfloat16, addr_space="Shared")
    s2m_slab = nc.dram_tensor("s2m_slab", [8], dt.float32)
    s2m_full = nc.dram_tensor("s2m_full", [8 * NCORES], dt.float32, addr_space="Shared")
    s2p_slab = nc.dram_tensor("s2p_slab", [SLAB], dt.float32)
    s2p_full = nc.dram_tensor("s2p_full", [N], dt.float32, addr_space="Shared")
    g2_slab = nc.dram_tensor("g2_slab", [SLAB, 2 * PAD2], dt.bfloat16)
    g2_full = nc.dram_tensor("g2_full", [N, 2 * PAD2], dt.bfloat16, addr_space="Shared")

    rg = [list(range(NCORES))]

    with tile.TileContext(nc) as tc:
      with tc.tile_pool(name="adjt", bufs=NJT) as p_adjt:
        # ---------------- phase 1: s2, w, h=x@W, G build + gathers ----------
        with (
            tc.tile_pool(name="xw", bufs=1) as p_xw,
            tc.tile_pool(name="small", bufs=1) as p_sm,
            tc.tile_pool(name="gtmp", bufs=1) as p_gt,
        ):
            xhi, xlo = [], []
            xTh_t = xTh_d.rearrange("(ft p) i -> ft p i", p=128)
            xTl_t = xTl_d.rearrange("(ft p) i -> ft p i", p=128)
            for ft in range(NFT):
                t = p_xw.tile([128, SLAB], dt.bfloat16, tag="x", name="x", bufs=12)
                nc.sync.dma_start(t[:], xTh_t[ft])
                xhi.append(t)
                t = p_xw.tile([128, SLAB], dt.bfloat16, tag="x", name="x", bufs=12)
                nc.sync.dma_start(t[:], xTl_t[ft])
                xlo.append(t)
            u6 = p_sm.tile([128, NFT, 8], dt.bfloat16, tag="u6", name="u6")
            nc.sync.dma_start(u6[:], U6_d.rearrange("(ft p) c -> p ft c", p=128))
            u3 = p_sm.tile([128, NFT, 8], dt.bfloat16, tag="u3", name="u3")
            nc.sync.dma_start(u3[:], U3_d.rearrange("(ft p) c -> p ft c", p=128))

            # s2 = x @ u (tiny matmuls), slab max, tiny AllGather
            s2_sb = []
            for h in range(NH):
                s2_sb.append(p_sm.tile([128, NIT], dt.float32, tag="s2",
                                       name="s2", bufs=NH))
            with tc.tile_pool(name="psS", bufs=2, space="PSUM") as ps_s:
                for it in range(NIT):
                    p6 = ps_s.tile([128, 8], dt.float32, tag="p6", name="p6", bufs=2)
                    p3 = ps_s.tile([128, 8], dt.float32, tag="p3", name="p3", bufs=2)
                    for ft in range(NFT):
                        xh = xhi[ft][:, it * 128:(it + 1) * 128]
                        xl = xlo[ft][:, it * 128:(it + 1) * 128]
                        nc.tensor.matmul(p6[:], xh, u6[:, ft, :],
                                         start=(ft == 0), stop=(ft == NFT - 1))
                        nc.tensor.matmul(p3[:], xl, u3[:, ft, :],
                                         start=(ft == 0), stop=(ft == NFT - 1))
                    t6 = p_sm.tile([128, 8], dt.float32, tag="t6",
                                   name="t6", bufs=2)
                    nc.vector.tensor_copy(t6[:], p6[:])
                    tsum = p_sm.tile([128, NH], dt.float32, tag="tsum",
                                     name="tsum", bufs=2)
                    nc.vector.tensor_tensor(tsum[:], t6[:, 0:2 * NH:2],
                                            t6[:, 1:2 * NH:2], ALU.add)
                    for h in range(NH):
                        nc.vector.tensor_tensor(s2_sb[h][:, it:it + 1],
                                                tsum[:, h:h + 1], p3[:, h:h + 1],
                                                ALU.add)

            sm8 = p_sm.tile([1, 8], dt.float32, tag="sm8", name="sm8")
            nc.vector.memset(sm8[:], 0.0)
            for h in range(NH):
                m1 = p_sm.tile([128, 1], dt.float32, tag="m1", name="m1", bufs=2)
                nc.vector.tensor_reduce(m1[:], s2_sb[h][:],
                                        axis=mybir.AxisListType.X, op=ALU.max)
                m2 = p_sm.tile([128, 1], dt.float32, tag="m2", name="m2", bufs=2)
                nc.gpsimd.partition_all_reduce(m2[:], m1[:], channels=128,
                                               reduce_op=bass_isa.ReduceOp.max)
                nc.vector.tensor_copy(sm8[0:1, h:h + 1], m2[0:1, 0:1])
            nc.sync.dma_start(s2m_slab[:].rearrange("(o a) -> o a", o=1), sm8[:])
            nc.gpsimd.collective_compute(
                "AllGather", ALU.bypass, replica_groups=rg,
                ins=[s2m_slab[:]], outs=[s2m_full[:]])

            # W + adjacency loads overlap the collective latency
            whi = [[None] * NFT for _ in range(NH)]
            wlo = [[None] * NFT for _ in range(NH)]
            Wh_t = Wh_d.rearrange("h (ft p) o -> h ft p o", p=128)
            Wl_t = Wl_d.rearrange("h (ft p) o -> h ft p o", p=128)
            for h in range(NH):
                for ft in range(NFT):
                    t = p_xw.tile([128, HID], dt.bfloat16, tag="w", name="w", bufs=36)
                    nc.sync.dma_start(t[:], Wh_t[h, ft])
                    whi[h][ft] = t
                    t = p_xw.tile([128, HID], dt.bfloat16, tag="w", name="w", bufs=36)
                    nc.scalar.dma_start(t[:], Wl_t[h, ft])
                    wlo[h][ft] = t
            adjt = []
            adjT_t = adjT_d.rearrange("(jt p) i -> jt p i", p=128)
            for j in range(NJT):
                t = p_adjt.tile([128, SLAB], dt.bfloat16, tag="adjt", name="adjt")
                eng = nc.sync if j % 2 == 0 else nc.scalar
                eng.dma_start(t[:], adjT_t[j])
                adjt.append(t)

            mload = p_sm.tile([1, 8 * NCORES], dt.float32, tag="mload", name="mload")
            nc.sync.dma_start(mload[:], s2m_full[:].rearrange("(o a) -> o a", o=1))
            negC = p_sm.tile([1, NH], dt.float32, tag="negC", name="negC")
            for h in range(NH):
                nc.vector.tensor_reduce(
                    negC[0:1, h:h + 1], mload[0:1, h::8],
                    axis=mybir.AxisListType.X, op=ALU.max, negate=True)
            negCbc = p_sm.tile([128, NH], dt.float32, tag="negCbc", name="negCbc")
            nc.gpsimd.partition_broadcast(negCbc[:], negC[:], channels=128)

            w_sb = []
            for h in range(NH):
                w = p_sm.tile([128, NIT], dt.float32, tag="wexp", name="wexp", bufs=NH)
                nc.scalar.activation(w[:], s2_sb[h][:], AF.Exp,
                                     bias=negCbc[:, h:h + 1])
                w_sb.append(w)
            # bf16 pair of the w columns -> gsw slab -> tiny gather
            whi3 = p_sm.tile([128, NH, NIT], dt.bfloat16, tag="whi3", name="whi3")
            wlo3 = p_sm.tile([128, NH, NIT], dt.float32, tag="wlo3", name="wlo3")
            wlo3b = p_sm.tile([128, NH, NIT], dt.bfloat16, tag="wlo3b", name="wlo3b")
            for h in range(NH):
                nc.vector.tensor_copy(whi3[:, h, :], w_sb[h][:])
                nc.vector.tensor_tensor(wlo3[:, h, :], w_sb[h][:], whi3[:, h, :],
                                        ALU.subtract)
            nc.vector.tensor_copy(wlo3b[:], wlo3[:])
            for it in range(NIT):
                wt = p_sm.tile([128, WCOLS], dt.bfloat16, tag="wt", name="wt", bufs=2)
                nc.vector.memset(wt[:], 0.0)
                nc.vector.tensor_copy(wt[:, 0:NH], whi3[:, :, it])
                nc.vector.tensor_copy(wt[:, 8:8 + NH], wlo3b[:, :, it])
                nc.sync.dma_start(gsw[it * 128:(it + 1) * 128, :], wt[:])
            nc.gpsimd.collective_compute(
                "AllGather", ALU.bypass, replica_groups=rg,
                ins=[gsw[:]], outs=[gfw[:]])

            # h = x@W per head; scale by w; bf16 pair; per-head gather
            with tc.tile_pool(name="psA", bufs=4, space="PSUM") as ps_a:
                for h in range(NH):
                    for it in range(NIT):
                        ps = ps_a.tile([128, HID], dt.float32, tag="psA", name="psA")
                        c0 = c1 = 0
                        for ft in range(NFT):
                            xh = xhi[ft][:, it * 128:(it + 1) * 128]
                            xl = xlo[ft][:, it * 128:(it + 1) * 128]
                            for lhs, rhss in ((xh, (whi[h][ft], wlo[h][ft])),
                                              (xl, (whi[h][ft],))):
                                for rhs in rhss:
                                    nc.tensor.matmul(
                                        ps[:, 0:512], lhs, rhs[:, 0:512],
                                        start=(c0 == 0), stop=(c0 == 3 * NFT - 1))
                                    c0 += 1
                                    nc.tensor.matmul(
                                        ps[:, 512:HID], lhs, rhs[:, 512:HID],
                                        start=(c1 == 0), stop=(c1 == 3 * NFT - 1))
                                    c1 += 1
                        g = p_gt.tile([128, HID], dt.float32, tag="g", name="g",
                                      bufs=3)
                        nc.vector.tensor_scalar_mul(g[:], ps[:],
                                                    w_sb[h][:, it:it + 1])
                        ghi = p_gt.tile([128, HID], dt.bfloat16, tag="ghi",
                                        name="ghi", bufs=3)
                        glo32 = p_gt.tile([128, HID], dt.float32, tag="glo32",
                                          name="glo32", bufs=3)
                        glo = p_gt.tile([128, HID], dt.bfloat16, tag="glo",
                                        name="glo", bufs=3)
                        nc.vector.tensor_copy(ghi[:], g[:])
                        nc.vector.tensor_tensor(glo32[:], g[:], ghi[:], ALU.subtract)
                        nc.vector.tensor_copy(glo[:], glo32[:])
                        rows = slice(it * 128, (it + 1) * 128)
                        nc.sync.dma_start(gs[h][rows, 0:HID], ghi[:])
                        nc.sync.dma_start(gs[h][rows, HID:2 * HID], glo[:])
                    nc.gpsimd.collective_compute(
                        "AllGather", ALU.bypass, replica_groups=rg,
                        ins=[gs[h][:]], outs=[gf[h][:]])

        # ---------------- L1 adjacency matmul + epilogue + layer 2 ----------
        with tc.tile_pool(name="xct", bufs=1) as p_xct:
            with (
                tc.tile_pool(name="numt", bufs=2) as p_numt,
                tc.tile_pool(name="gst", bufs=12) as p_gst,
                tc.tile_pool(name="etmp", bufs=1) as p_et,
                tc.tile_pool(name="wo", bufs=1) as p_wo,
                tc.tile_pool(name="l2a", bufs=1) as p_l2a,
                tc.tile_pool(name="ps1", bufs=4, space="PSUM") as ps_1,
                tc.tile_pool(name="psh2", bufs=4, space="PSUM") as ps_h2,
            ):
                # denominator col-tile first: den_k = A @ w_k
                gwv = gfw.rearrange("(jt p) c -> p jt c", p=128)
                gwt = p_gst.tile([128, NJT, WCOLS], dt.bfloat16, tag="gwt",
                                 name="gwt", bufs=1)
                nc.sync.dma_start(gwt[:], gwv[:])
                psd = ps_1.tile([128, SLAB], dt.float32, tag="ps1", name="ps1")
                for j in range(NJT):
                    nc.tensor.matmul(psd[0:NH, :], gwt[:, j, 0:NH], adjt[j][:],
                                     start=(j == 0), stop=False)
                    nc.tensor.matmul(psd[0:NH, :], gwt[:, j, 8:8 + NH], adjt[j][:],
                                     start=False, stop=(j == NJT - 1))
                recip3 = p_et.tile([NH, SLAB], dt.float32, tag="recip3",
                                   name="recip3")
                nc.vector.reciprocal(recip3[:], psd[0:NH, :])
                rbc = []
                for h in range(NH):
                    rrow = p_et.tile([1, SLAB], dt.float32, tag="rrow",
                                     name="rrow", bufs=2)
                    nc.sync.dma_start(rrow[:], recip3[h:h + 1, :])
                    rb = p_et.tile([128, SLAB], dt.float32, tag="rbc",
                                   name="rbc", bufs=NH)
                    nc.gpsimd.partition_broadcast(rb[:], rrow[:], channels=128)
                    rbc.append(rb)

                # feature col-tiles, head-major; epilogue inline per ct
                xchi, xclo = [], []
                for ct in range(NCT):
                    h = ct // NFT
                    lct = ct % NFT
                    gv = gf[h].rearrange("(jb q p) (t c) -> jb p q t c",
                                         q=4, p=128, t=2)
                    ps = ps_1.tile([128, SLAB], dt.float32, tag="ps1", name="ps1")
                    for jb in range(NJT // 4):
                        gt = p_gst.tile([128, 4, 2, 128], dt.bfloat16, tag="gst",
                                        name="gst")
                        for tt in range(2):
                            eng = nc.sync if (jb + tt) % 2 == 0 else nc.scalar
                            eng.dma_start(gt[:, :, tt, :],
                                          gv[jb, :, :, tt,
                                             lct * 128:(lct + 1) * 128])
                        for q in range(4):
                            j = jb * 4 + q
                            nc.tensor.matmul(ps[:], gt[:, q, 0, :], adjt[j][:],
                                             start=(j == 0), stop=False)
                            nc.tensor.matmul(ps[:], gt[:, q, 1, :], adjt[j][:],
                                             start=False, stop=(j == NJT - 1))
                    # xcatT tile = elu(numT / den) and its bf16 pair
                    z = p_et.tile([128, SLAB], dt.float32, tag="z", name="z", bufs=2)
                    nc.vector.tensor_tensor(z[:], ps[:], rbc[h][:], ALU.mult)
                    e = p_et.tile([128, SLAB], dt.float32, tag="e", name="e", bufs=2)
                    nc.scalar.activation(e[:], z[:], AF.Exp)
                    nc.vector.tensor_scalar(e[:], e[:], 1.0, -1.0, ALU.min, ALU.add)
                    xc = p_et.tile([128, SLAB], dt.float32, tag="xc", name="xc",
                                   bufs=2)
                    nc.vector.scalar_tensor_tensor(xc[:], z[:], 0.0, e[:],
                                                   ALU.max, ALU.add)
                    th = p_xct.tile([128, SLAB], dt.bfloat16, tag="xcp",
                                    name="xcp", bufs=2 * NCT)
                    nc.vector.tensor_copy(th[:], xc[:])
                    r32 = p_et.tile([128, SLAB], dt.float32, tag="r32",
                                    name="r32", bufs=2)
                    nc.vector.tensor_tensor(r32[:], xc[:], th[:], ALU.subtract)
                    tl = p_xct.tile([128, SLAB], dt.bfloat16, tag="xcp",
                                    name="xcp", bufs=2 * NCT)
                    nc.vector.tensor_copy(tl[:], r32[:])
                    xchi.append(th)
                    xclo.append(tl)

                # Wo loads (needed only by the h2 matmul below; emitted
                # mid-ct-loop region so they don't clog the queues that feed
                # the L1 stationary prefetch)
                woh, wol = [], []
                Woh_t = Woh_d.rearrange("(ot p) c -> ot p c", p=128)
                Wol_t = Wol_d.rearrange("(ot p) c -> ot p c", p=128)
                for ot in range(NCT):
                    t = p_wo.tile([128, G2C], dt.bfloat16, tag="wo", name="wo",
                                  bufs=2 * NCT)
                    nc.sync.dma_start(t[:], Woh_t[ot])
                    woh.append(t)
                    t = p_wo.tile([128, G2C], dt.bfloat16, tag="wo", name="wo",
                                  bufs=2 * NCT)
                    nc.scalar.dma_start(t[:], Wol_t[ot])
                    wol.append(t)
                # layer 2: h2 = xcat@Wo (+ s2' via folded u2 column)
                ps2l = [ps_h2.tile([128, G2C], dt.float32, tag="psh2",
                                   name="psh2") for _ in range(NIT)]
                for ot in range(NCT):
                    for it in range(NIT):
                        xh = xchi[ot][:, it * 128:(it + 1) * 128]
                        xl = xclo[ot][:, it * 128:(it + 1) * 128]
                        nc.tensor.matmul(ps2l[it][:], xh, woh[ot][:],
                                         start=(ot == 0), stop=False)
                        nc.tensor.matmul(ps2l[it][:], xh, wol[ot][:],
                                         start=False, stop=False)
                        nc.tensor.matmul(ps2l[it][:], xl, woh[ot][:],
                                         start=False, stop=(ot == NCT - 1))
                h2_sb = []
                s2p = p_l2a.tile([128, NIT], dt.float32, tag="s2p", name="s2p")
                for it in range(NIT):
                    h2 = p_l2a.tile([128, NCLS], dt.float32, tag="h2", name="h2",
                                    bufs=NIT)
                    nc.vector.tensor_copy(h2[:], ps2l[it][:, 0:NCLS])
                    h2_sb.append(h2)
                    nc.vector.tensor_copy(s2p[:, it:it + 1],
                                          ps2l[it][:, NCLS:G2C])
                    nc.sync.dma_start(
                        s2p_slab[it * 128:(it + 1) * 128].rearrange(
                            "(p o) -> p o", o=1),
                        s2p[:, it:it + 1])
                nc.gpsimd.collective_compute(
                    "AllGather", ALU.bypass, replica_groups=rg,
                    ins=[s2p_slab[:]], outs=[s2p_full[:]])
                sload = p_l2a.tile([128, N // 128], dt.float32, tag="sload",
                                   name="sload")
                nc.sync.dma_start(sload[:],
                                  s2p_full[:].rearrange("(p a) -> p a", p=128))
                sm1 = p_l2a.tile([128, 1], dt.float32, tag="sm1", name="sm1")
                nc.vector.tensor_reduce(sm1[:], sload[:],
                                        axis=mybir.AxisListType.X, op=ALU.max)
                sm2 = p_l2a.tile([128, 1], dt.float32, tag="sm2", name="sm2")
                nc.gpsimd.partition_all_reduce(sm2[:], sm1[:], channels=128,
                                               reduce_op=bass_isa.ReduceOp.max)
                negC2 = p_l2a.tile([128, 1], dt.float32, tag="negC2", name="negC2")
                nc.vector.tensor_scalar_mul(negC2[:], sm2[:], -1.0)
                w2all = p_l2a.tile([128, NIT], dt.float32, tag="w2all", name="w2all")
                nc.scalar.activation(w2all[:], s2p[:], AF.Exp, bias=negC2[:])
                for it in range(NIT):
                    rows = slice(it * 128, (it + 1) * 128)
                    g2 = p_l2a.tile([128, PAD2], dt.float32, tag="g2", name="g2",
                                    bufs=2)
                    nc.vector.tensor_scalar_mul(g2[:, 0:NCLS], h2_sb[it][:],
                                                w2all[:, it:it + 1])
                    nc.vector.tensor_copy(g2[:, NCLS:G2C], w2all[:, it:it + 1])
                    nc.vector.memset(g2[:, G2C:PAD2], 0.0)
                    g2h = p_l2a.tile([128, PAD2], dt.bfloat16, tag="g2h",
                                     name="g2h", bufs=2)
                    g2r = p_l2a.tile([128, PAD2], dt.float32, tag="g2r",
                                     name="g2r", bufs=2)
                    g2l = p_l2a.tile([128, PAD2], dt.bfloat16, tag="g2l",
                                     name="g2l", bufs=2)
                    nc.vector.tensor_copy(g2h[:], g2[:])
                    nc.vector.tensor_tensor(g2r[:], g2[:], g2h[:], ALU.subtract)
                    nc.vector.tensor_copy(g2l[:], g2r[:])
                    nc.sync.dma_start(g2_slab[rows, 0:PAD2], g2h[:])
                    nc.sync.dma_start(g2_slab[rows, PAD2:2 * PAD2], g2l[:])
                nc.gpsimd.collective_compute(
                    "AllGather", ALU.bypass, replica_groups=rg,
                    ins=[g2_slab[:]], outs=[g2_full[:]])

            # L2 adjacency matmul + final epilogue
            with (
                tc.tile_pool(name="g2t", bufs=NJT) as p_g2t,
                tc.tile_pool(name="fin", bufs=1) as p_f,
                tc.tile_pool(name="ps2", bufs=4, space="PSUM") as ps_2,
            ):
                g2v = g2_full.rearrange("(jt p) (t c) -> jt p t c", p=128, t=2)
                g2tiles = []
                for j in range(NJT):
                    gt2 = p_g2t.tile([128, 2, PAD2], dt.bfloat16, tag="g2t",
                                     name="g2t")
                    eng = nc.sync if j % 2 == 0 else nc.scalar
                    eng.dma_start(gt2[:, :, 0:G2C], g2v[j, :, :, 0:G2C])
                    g2tiles.append(gt2)
                ps2 = [ps_2.tile([128, G2C], dt.float32, tag="ps2", name="ps2")
                       for _ in range(NIT)]
                for it in range(NIT):
                    for j in range(NJT):
                        lhs = adjt[j][:, it * 128:(it + 1) * 128]
                        nc.tensor.matmul(ps2[it][:], lhs,
                                         g2tiles[j][:, 0, 0:G2C],
                                         start=(j == 0), stop=False)
                        nc.tensor.matmul(ps2[it][:], lhs,
                                         g2tiles[j][:, 1, 0:G2C],
                                         start=False, stop=(j == NJT - 1))
                for it in range(NIT):
                    r2 = p_f.tile([128, 1], dt.float32, tag="r2", name="r2", bufs=2)
                    nc.vector.reciprocal(r2[:], ps2[it][:, NCLS:G2C])
                    z = p_f.tile([128, NCLS], dt.float32, tag="z2", name="z2",
                                 bufs=2)
                    nc.vector.tensor_scalar_mul(z[:], ps2[it][:, 0:NCLS], r2[:])
                    e = p_f.tile([128, NCLS], dt.float32, tag="e2", name="e2",
                                 bufs=2)
                    nc.scalar.activation(e[:], z[:], AF.Exp)
                    nc.vector.tensor_scalar(e[:], e[:], 1.0, -1.0, ALU.min, ALU.add)
                    o = p_f.tile([128, NCLS], dt.float32, tag="o2", name="o2",
                                 bufs=2)
                    nc.vector.scalar_tensor_tensor(o[:], z[:], 0.0, e[:],
                                                   ALU.max, ALU.add)
                    negm = p_f.tile([128, 1], dt.float32, tag="negm", name="negm",
                                    bufs=2)
                    nc.vector.tensor_reduce(negm[:], o[:],
                                            axis=mybir.AxisListType.X,
                                            op=ALU.max, negate=True)
                    t = p_f.tile([128, NCLS], dt.float32, tag="texp", name="texp",
                                 bufs=2)
                    nc.scalar.activation(t[:], o[:], AF.Exp, bias=negm[:])
                    ssum = p_f.tile([128, 1], dt.float32, tag="ssum", name="ssum",
                                    bufs=2)
                    nc.vector.tensor_reduce(ssum[:], t[:],
                                            axis=mybir.AxisListType.X, op=ALU.add)
                    lg = p_f.tile([128, 1], dt.float32, tag="lg", name="lg", bufs=2)
                    nc.scalar.activation(lg[:], ssum[:], AF.Ln)
                    fin = p_f.tile([128, NCLS], dt.float32, tag="fin", name="fin",
                                   bufs=2)
                    nc.vector.tensor_scalar(fin[:], o[:], negm[:], lg[:],
                                            ALU.add, ALU.subtract)
                    nc.sync.dma_start(out_d[it * 128:(it + 1) * 128, :], fin[:])

    nc.finalize()
    return nc


_CACHE = {}


def _pair(a):
    hi = a.astype(BF16)
    lo = (a - hi.astype(np.float32)).astype(BF16)
    return hi, lo


def prepare_inputs(x, adj, W_heads, a_heads, W_out, a_out):
    """Shard + lay out the full inputs for the 8 cores."""
    x2 = np.asarray(x, np.float32)[0]          # [N, F]
    adj2 = np.asarray(adj)[0]                  # [N, N] int32
    W3 = np.asarray(W_heads, np.float32).reshape(NH, F, HID)
    a3 = np.asarray(a_heads, np.float32)       # [NH, 2*HID, 1]
    Wo = np.asarray(W_out, np.float32).reshape(GH, NCLS)
    ao = np.asarray(a_out, np.float32)         # [2*NCLS, 1]

    # fold the edge-score projections into the weights:
    #   s2 = x @ (W @ a2),   s2' = xcat @ (Wo @ ao2)
    u = np.einsum("hfo,ho->hf", W3.astype(np.float64),
                  a3[:, HID:, 0].astype(np.float64)).astype(np.float32)  # [NH,F]
    u_hi, u_lo = _pair(u)
    U6 = np.zeros((F, 8), BF16)
    U3 = np.zeros((F, 8), BF16)
    for h in range(NH):
        U6[:, 2 * h] = u_hi[h]
        U6[:, 2 * h + 1] = u_lo[h]
        U3[:, h] = u_hi[h]
    u2 = (Wo.astype(np.float64) @ ao[NCLS:, 0].astype(np.float64)).astype(np.float32)
    Wo_ext = np.concatenate([Wo, u2[:, None]], axis=1)       # [GH, 257]
    Woh, Wol = _pair(Wo_ext)
    Wh, Wl = _pair(W3)
    xT = np.ascontiguousarray(x2.T)            # [F, N]
    adjb = adj2.astype(BF16)                   # exact 0/1

    in_maps = []
    for c in range(NCORES):
        sl = slice(c * SLAB, (c + 1) * SLAB)
        xh, xl = _pair(np.ascontiguousarray(xT[:, sl]))
        in_maps.append({
            "adjT": np.ascontiguousarray(adjb[sl, :].T),
            "xT_hi": xh, "xT_lo": xl,
            "U6": U6, "U3": U3,
            "W_hi": Wh, "W_lo": Wl,
            "Wo_hi": Woh, "Wo_lo": Wol,
        })
    return in_maps


def kernel(x, adj, W_heads, a_heads, W_out, a_out):
    if "nc" not in _CACHE:
        # touch the devices once so any residual bad state from a previous
        # process surfaces (and clears) before the real run
        try:
            import jax
            jax.block_until_ready(jax.numpy.zeros(8))
        except Exception:
            pass
        _CACHE["nc"] = build()
    nc = _CACHE["nc"]
    in_maps = prepare_inputs(x, adj, W_heads, a_heads, W_out, a_out)
    res = run_bass_kernel_spmd(nc, in_maps, list(range(NCORES)))
    out = np.concatenate([res.results[c]["out"] for c in range(NCORES)], axis=0)
    return out.reshape(1, N, NCLS)



# revision 4
# speedup vs baseline: 1.6456x; 1.6456x over previous
"""GAT (2-layer, 3-head) forward on 8 Trainium2 NeuronCores.

Math: with LeakyReLU slope ALPHA=1.0 the edge score e_ij = s1_i + s2_j is
linear, and s1_i cancels inside the row softmax.  The masked softmax over
j therefore reduces to column weights w_j = exp(s2_j - C) restricted to
adj, giving

    h'_i = (sum_j adj_ij * w_j * h_j) / (sum_j adj_ij * w_j)

i.e. one adjacency matmul against G = [w*h | w].  Both GAT layers take
this form (the same adjacency masks both), so the whole network is two
A-matmuls plus small projections.

Sharding: rows of h' (nodes) across 8 cores; each core holds lhsT-layout
adjacency columns A^T[:, slab] and computes its 512-row slab.  Matmuls
run in single bf16 (the tolerance is 2e-2; only the edge-score s2, which
sits in an exponent, is kept in ~fp32 via a hi/lo pair trick folded into
the input prep).  G is gathered in two chunks: [G0 | w] first so the
denominator and head-0 tiles unblock early, then [G1 | G2].
"""
import sys

sys.path.insert(0, "/opt/trn_rl_repo")

import numpy as np
import ml_dtypes

import concourse.bass as bass
import concourse.bacc as bacc
import concourse.mybir as mybir
import concourse.bass_isa as bass_isa
import concourse.tile as tile
from concourse.bass_utils import run_bass_kernel_spmd

BF16 = ml_dtypes.bfloat16

N = 4096
F = 768
HID = 768
NH = 3
NCLS = 256
NCORES = 8
SLAB = N // NCORES          # 512 rows per core
NIT = SLAB // 128           # 4 i-tiles per core
NJT = N // 128              # 32 j-tiles
NFT = F // 128              # 6 f-tiles
NCT = NH * NFT              # 18 feature col-tiles of G
G2C = NCLS + 1              # 257 = classes + s2' column (folded u2)
PAD2 = 264                  # G2 padded to 32B rows
WCOLS = 16                  # w-column slab width (3 used + pad)
GA = HID + WCOLS            # gather-A width: head0 G + w columns
GB = 2 * HID                # gather-B width: head1 + head2 G
GH_TOT = NH * HID           # 2304 xcat feature rows of Wo

AF = mybir.ActivationFunctionType
ALU = mybir.AluOpType


def _enable_ldw_opt():
    # walrus defaults to --enable-ldw-opt=false; with it off every LDWEIGHTS
    # serializes against the previous matmul (~427ns vs ~213ns per 512-col
    # matmul).  Patch the arg builder so the stationary loads pipeline.
    import concourse.bass_utils as _bu
    if getattr(_bu, "_ldw_opt_patched", False):
        return
    _orig = _bu.get_walrus_args

    def _patched(*a, **k):
        args = _orig(*a, **k)
        return [x.replace("--enable-ldw-opt=false", "--enable-ldw-opt=true")
                for x in args]

    _bu.get_walrus_args = _patched
    _bu._ldw_opt_patched = True


def build():
    dt = mybir.dt
    _enable_ldw_opt()
    nc = bacc.Bacc(num_devices=NCORES)

    adjT_d = nc.dram_tensor("adjT", [N, SLAB], dt.bfloat16, kind="ExternalInput")
    xTh_d = nc.dram_tensor("xT_hi", [F, SLAB], dt.bfloat16, kind="ExternalInput")
    xTl_d = nc.dram_tensor("xT_lo", [F, SLAB], dt.bfloat16, kind="ExternalInput")
    U6_d = nc.dram_tensor("U6", [F, 8], dt.bfloat16, kind="ExternalInput")
    U3_d = nc.dram_tensor("U3", [F, 8], dt.bfloat16, kind="ExternalInput")
    W_d = nc.dram_tensor("W", [NH, F, HID], dt.bfloat16, kind="ExternalInput")
    Wo_d = nc.dram_tensor("Wo", [GH_TOT, G2C], dt.bfloat16, kind="ExternalInput")
    out_d = nc.dram_tensor("out", [SLAB, NCLS], dt.float32, kind="ExternalOutput")

    # DRAM scratch + collective buffers
    gsA = nc.dram_tensor("gsA", [SLAB, GA], dt.bfloat16)
    gfA = nc.dram_tensor("gfA", [N, GA], dt.bfloat16, addr_space="Shared")
    gsB = nc.dram_tensor("gsB", [SLAB, GB], dt.bfloat16)
    gfB = nc.dram_tensor("gfB", [N, GB], dt.bfloat16, addr_space="Shared")
    s2m_slab = nc.dram_tensor("s2m_slab", [8], dt.float32)
    s2m_full = nc.dram_tensor("s2m_full", [8 * NCORES], dt.float32, addr_space="Shared")
    s2p_slab = nc.dram_tensor("s2p_slab", [SLAB], dt.float32)
    s2p_full = nc.dram_tensor("s2p_full", [N], dt.float32, addr_space="Shared")
    g2_slab = nc.dram_tensor("g2_slab", [SLAB, PAD2], dt.bfloat16)
    g2_full = nc.dram_tensor("g2_full", [N, PAD2], dt.bfloat16, addr_space="Shared")

    rg = [list(range(NCORES))]

    with tile.TileContext(nc) as tc:
      with tc.tile_pool(name="adjt", bufs=NJT) as p_adjt:
        # ---------------- phase 1: s2, w, h=x@W, G build + gathers ----------
        with (
            tc.tile_pool(name="xw", bufs=1) as p_xw,
            tc.tile_pool(name="small", bufs=1) as p_sm,
            tc.tile_pool(name="gtmp", bufs=1) as p_gt,
        ):
            # x tiles first (s2 needs all of them), W next (x@W starts as
            # soon as head-0 W lands), adjacency last.
            xhi, xlo = [], []
            xTh_t = xTh_d.rearrange("(ft p) i -> ft p i", p=128)
            xTl_t = xTl_d.rearrange("(ft p) i -> ft p i", p=128)
            for ft in range(NFT):
                t = p_xw.tile([128, SLAB], dt.bfloat16, tag="x", name="x", bufs=12)
                nc.sync.dma_start(t[:], xTh_t[ft])
                xhi.append(t)
                t = p_xw.tile([128, SLAB], dt.bfloat16, tag="x", name="x", bufs=12)
                nc.scalar.dma_start(t[:], xTl_t[ft])
                xlo.append(t)
            u6 = p_sm.tile([128, NFT, 8], dt.bfloat16, tag="u6", name="u6")
            nc.sync.dma_start(u6[:], U6_d.rearrange("(ft p) c -> p ft c", p=128))
            u3 = p_sm.tile([128, NFT, 8], dt.bfloat16, tag="u3", name="u3")
            nc.scalar.dma_start(u3[:], U3_d.rearrange("(ft p) c -> p ft c", p=128))

            wt_sb = [[None] * NFT for _ in range(NH)]
            W_t = W_d.rearrange("h (ft p) o -> h ft p o", p=128)
            for h in range(NH):
                for ft in range(NFT):
                    t = p_xw.tile([128, HID], dt.bfloat16, tag="w", name="w",
                                  bufs=NH * NFT)
                    eng = nc.sync if (h * NFT + ft) % 2 == 0 else nc.scalar
                    eng.dma_start(t[:], W_t[h, ft])
                    wt_sb[h][ft] = t
            adjt = []
            adjT_t = adjT_d.rearrange("(jt p) i -> jt p i", p=128)
            for j in range(NJT):
                t = p_adjt.tile([128, SLAB], dt.bfloat16, tag="adjt", name="adjt")
                eng = nc.sync if j % 2 == 0 else nc.scalar
                eng.dma_start(t[:], adjT_t[j])
                adjt.append(t)

            # s2 = x @ u (tiny matmuls, hi/lo pair precision), slab max,
            # tiny AllGather.
            s2_sb = []
            for h in range(NH):
                s2_sb.append(p_sm.tile([128, NIT], dt.float32, tag="s2",
                                       name="s2", bufs=NH))
            with tc.tile_pool(name="psS", bufs=2, space="PSUM") as ps_s:
                for it in range(NIT):
                    p6 = ps_s.tile([128, 8], dt.float32, tag="p6", name="p6", bufs=2)
                    p3 = ps_s.tile([128, 8], dt.float32, tag="p3", name="p3", bufs=2)
                    for ft in range(NFT):
                        xh = xhi[ft][:, it * 128:(it + 1) * 128]
                        xl = xlo[ft][:, it * 128:(it + 1) * 128]
                        nc.tensor.matmul(p6[:], xh, u6[:, ft, :],
                                         start=(ft == 0), stop=(ft == NFT - 1))
                        nc.tensor.matmul(p3[:], xl, u3[:, ft, :],
                                         start=(ft == 0), stop=(ft == NFT - 1))
                    t6 = p_sm.tile([128, 8], dt.float32, tag="t6",
                                   name="t6", bufs=2)
                    nc.vector.tensor_copy(t6[:], p6[:])
                    tsum = p_sm.tile([128, NH], dt.float32, tag="tsum",
                                     name="tsum", bufs=2)
                    nc.vector.tensor_tensor(tsum[:], t6[:, 0:2 * NH:2],
                                            t6[:, 1:2 * NH:2], ALU.add)
                    for h in range(NH):
                        nc.vector.tensor_tensor(s2_sb[h][:, it:it + 1],
                                                tsum[:, h:h + 1], p3[:, h:h + 1],
                                                ALU.add)

            sm8 = p_sm.tile([1, 8], dt.float32, tag="sm8", name="sm8")
            nc.vector.memset(sm8[:], 0.0)
            for h in range(NH):
                m1 = p_sm.tile([128, 1], dt.float32, tag="m1", name="m1", bufs=2)
                nc.vector.tensor_reduce(m1[:], s2_sb[h][:],
                                        axis=mybir.AxisListType.X, op=ALU.max)
                m2 = p_sm.tile([128, 1], dt.float32, tag="m2", name="m2", bufs=2)
                nc.gpsimd.partition_all_reduce(m2[:], m1[:], channels=128,
                                               reduce_op=bass_isa.ReduceOp.max)
                nc.vector.tensor_copy(sm8[0:1, h:h + 1], m2[0:1, 0:1])
            nc.sync.dma_start(s2m_slab[:].rearrange("(o a) -> o a", o=1), sm8[:])
            nc.gpsimd.collective_compute(
                "AllGather", ALU.bypass, replica_groups=rg,
                ins=[s2m_slab[:]], outs=[s2m_full[:]])

            # h = x@W per head while the tiny collective is in flight;
            # park h tiles in SBUF fp32 until w is ready.
            h_sb = [[None] * NIT for _ in range(NH)]
            with tc.tile_pool(name="psA", bufs=2, space="PSUM") as ps_a:
                for h in range(NH):
                    for it in range(NIT):
                        ps = ps_a.tile([128, HID], dt.float32, tag="psA", name="psA")
                        for ft in range(NFT):
                            xh = xhi[ft][:, it * 128:(it + 1) * 128]
                            w = wt_sb[h][ft]
                            nc.tensor.matmul(ps[:, 0:512], xh, w[:, 0:512],
                                             start=(ft == 0), stop=(ft == NFT - 1))
                            nc.tensor.matmul(ps[:, 512:HID], xh, w[:, 512:HID],
                                             start=(ft == 0), stop=(ft == NFT - 1))
                        ht = p_gt.tile([128, HID], dt.float32, tag="h", name="h",
                                       bufs=NH * NIT)
                        nc.vector.tensor_copy(ht[:], ps[:])
                        h_sb[h][it] = ht

            # negC from the gathered per-core maxima
            mload = p_sm.tile([1, 8 * NCORES], dt.float32, tag="mload", name="mload")
            nc.sync.dma_start(mload[:], s2m_full[:].rearrange("(o a) -> o a", o=1))
            negC = p_sm.tile([1, NH], dt.float32, tag="negC", name="negC")
            for h in range(NH):
                nc.vector.tensor_reduce(
                    negC[0:1, h:h + 1], mload[0:1, h::8],
                    axis=mybir.AxisListType.X, op=ALU.max, negate=True)
            negCbc = p_sm.tile([128, NH], dt.float32, tag="negCbc", name="negCbc")
            nc.gpsimd.partition_broadcast(negCbc[:], negC[:], channels=128)

            w_sb = []
            for h in range(NH):
                w = p_sm.tile([128, NIT], dt.float32, tag="wexp", name="wexp",
                              bufs=NH)
                nc.scalar.activation(w[:], s2_sb[h][:], AF.Exp,
                                     bias=negCbc[:, h:h + 1])
                w_sb.append(w)

            # gather A: [G0 | w columns] — unblocks denominator + head-0 tiles
            wb3 = p_sm.tile([128, NH, NIT], dt.bfloat16, tag="wb3", name="wb3")
            for h in range(NH):
                nc.vector.tensor_copy(wb3[:, h, :], w_sb[h][:])
            for it in range(NIT):
                rows = slice(it * 128, (it + 1) * 128)
                g = p_gt.tile([128, HID], dt.bfloat16, tag="g0", name="g0", bufs=2)
                nc.vector.tensor_scalar_mul(g[:], h_sb[0][it][:],
                                            w_sb[0][:, it:it + 1])
                nc.sync.dma_start(gsA[rows, 0:HID], g[:])
                wt = p_sm.tile([128, WCOLS], dt.bfloat16, tag="wt", name="wt",
                               bufs=2)
                nc.vector.memset(wt[:], 0.0)
                nc.vector.tensor_copy(wt[:, 0:NH], wb3[:, :, it])
                nc.scalar.dma_start(gsA[rows, HID:GA], wt[:])
            nc.gpsimd.collective_compute(
                "AllGather", ALU.bypass, replica_groups=rg,
                ins=[gsA[:]], outs=[gfA[:]])

            # gather B: [G1 | G2]
            for it in range(NIT):
                rows = slice(it * 128, (it + 1) * 128)
                for h in (1, 2):
                    g = p_gt.tile([128, HID], dt.bfloat16, tag="g0", name="g0",
                                  bufs=2)
                    nc.vector.tensor_scalar_mul(g[:], h_sb[h][it][:],
                                                w_sb[h][:, it:it + 1])
                    eng = nc.sync if h == 1 else nc.scalar
                    eng.dma_start(gsB[rows, (h - 1) * HID:h * HID], g[:])
            nc.gpsimd.collective_compute(
                "AllGather", ALU.bypass, replica_groups=rg,
                ins=[gsB[:]], outs=[gfB[:]])

        # ---------------- L1 adjacency matmul + epilogue + layer 2 ----------
        with tc.tile_pool(name="xct", bufs=1) as p_xct:
            with (
                tc.tile_pool(name="gst", bufs=8) as p_gst,
                tc.tile_pool(name="etmp", bufs=1) as p_et,
                tc.tile_pool(name="wo", bufs=1) as p_wo,
                tc.tile_pool(name="l2a", bufs=1) as p_l2a,
                tc.tile_pool(name="ps1", bufs=4, space="PSUM") as ps_1,
                tc.tile_pool(name="psh2", bufs=4, space="PSUM") as ps_h2,
            ):
                # Wo loads early; they only feed the inline h2 matmuls
                wo_sb = []
                Wo_t = Wo_d.rearrange("(ot p) c -> ot p c", p=128)
                for ot in range(NCT):
                    t = p_wo.tile([128, G2C], dt.bfloat16, tag="wo", name="wo",
                                  bufs=NCT)
                    eng = nc.sync if ot % 2 == 0 else nc.scalar
                    eng.dma_start(t[:], Wo_t[ot])
                    wo_sb.append(t)

                # denominator col-tile first: den_k = A @ w_k
                gwv = gfA.rearrange("(jt p) c -> p jt c", p=128)
                gwt = p_gst.tile([128, NJT, WCOLS], dt.bfloat16, tag="gwt",
                                 name="gwt", bufs=1)
                nc.sync.dma_start(gwt[:], gwv[:, :, HID:GA])
                psd = ps_1.tile([128, SLAB], dt.float32, tag="ps1", name="ps1")
                for j in range(NJT):
                    nc.tensor.matmul(psd[0:NH, :], gwt[:, j, 0:NH], adjt[j][:],
                                     start=(j == 0), stop=(j == NJT - 1))
                recip3 = p_et.tile([NH, SLAB], dt.float32, tag="recip3",
                                   name="recip3")
                nc.vector.reciprocal(recip3[:], psd[0:NH, :])
                rbc = []
                for h in range(NH):
                    rrow = p_et.tile([1, SLAB], dt.float32, tag="rrow",
                                     name="rrow", bufs=2)
                    nc.sync.dma_start(rrow[:], recip3[h:h + 1, :])
                    rb = p_et.tile([128, SLAB], dt.float32, tag="rbc",
                                   name="rbc", bufs=NH)
                    nc.gpsimd.partition_broadcast(rb[:], rrow[:], channels=128)
                    rbc.append(rb)

                # feature col-tiles, head-major; epilogue + h2 inline per ct
                gvA = gfA.rearrange("(jb q p) c -> jb p q c", q=4, p=128)
                gvB = gfB.rearrange("(jb q p) c -> jb p q c", q=4, p=128)
                ps2l = [ps_h2.tile([128, G2C], dt.float32, tag="psh2",
                                   name="psh2") for _ in range(NIT)]
                for ct in range(NCT):
                    h = ct // NFT
                    lct = ct % NFT
                    ps = ps_1.tile([128, SLAB], dt.float32, tag="ps1", name="ps1")
                    for jb in range(NJT // 4):
                        gt = p_gst.tile([128, 4, 128], dt.bfloat16, tag="gst",
                                        name="gst")
                        eng = nc.sync if jb % 2 == 0 else nc.scalar
                        if h == 0:
                            eng.dma_start(gt[:], gvA[jb, :, :,
                                                     lct * 128:(lct + 1) * 128])
                        else:
                            c0 = (h - 1) * HID + lct * 128
                            eng.dma_start(gt[:], gvB[jb, :, :, c0:c0 + 128])
                        for q in range(4):
                            j = jb * 4 + q
                            nc.tensor.matmul(ps[:], gt[:, q, :], adjt[j][:],
                                             start=(j == 0), stop=(j == NJT - 1))
                    # xcatT tile = elu(numT / den), bf16
                    z = p_et.tile([128, SLAB], dt.float32, tag="z", name="z",
                                  bufs=2)
                    nc.vector.tensor_tensor(z[:], ps[:], rbc[h][:], ALU.mult)
                    e = p_et.tile([128, SLAB], dt.float32, tag="e", name="e",
                                  bufs=2)
                    nc.scalar.activation(e[:], z[:], AF.Exp)
                    nc.vector.tensor_scalar(e[:], e[:], 1.0, -1.0, ALU.min,
                                            ALU.add)
                    xc = p_xct.tile([128, SLAB], dt.bfloat16, tag="xcp",
                                    name="xcp", bufs=NCT)
                    nc.vector.scalar_tensor_tensor(xc[:], z[:], 0.0, e[:],
                                                   ALU.max, ALU.add)
                    # layer 2 accumulation: h2 += xcat_ct @ Wo_ct
                    for it in range(NIT):
                        nc.tensor.matmul(ps2l[it][:],
                                         xc[:, it * 128:(it + 1) * 128],
                                         wo_sb[ct][:],
                                         start=(ct == 0), stop=(ct == NCT - 1))

                h2_sb = []
                s2p = p_l2a.tile([128, NIT], dt.float32, tag="s2p", name="s2p")
                for it in range(NIT):
                    h2 = p_l2a.tile([128, NCLS], dt.float32, tag="h2", name="h2",
                                    bufs=NIT)
                    nc.vector.tensor_copy(h2[:], ps2l[it][:, 0:NCLS])
                    h2_sb.append(h2)
                    nc.vector.tensor_copy(s2p[:, it:it + 1],
                                          ps2l[it][:, NCLS:G2C])
                    nc.sync.dma_start(
                        s2p_slab[it * 128:(it + 1) * 128].rearrange(
                            "(p o) -> p o", o=1),
                        s2p[:, it:it + 1])
                nc.gpsimd.collective_compute(
                    "AllGather", ALU.bypass, replica_groups=rg,
                    ins=[s2p_slab[:]], outs=[s2p_full[:]])
                sload = p_l2a.tile([128, N // 128], dt.float32, tag="sload",
                                   name="sload")
                nc.sync.dma_start(sload[:],
                                  s2p_full[:].rearrange("(p a) -> p a", p=128))
                sm1 = p_l2a.tile([128, 1], dt.float32, tag="sm1", name="sm1")
                nc.vector.tensor_reduce(sm1[:], sload[:],
                                        axis=mybir.AxisListType.X, op=ALU.max)
                sm2 = p_l2a.tile([128, 1], dt.float32, tag="sm2", name="sm2")
                nc.gpsimd.partition_all_reduce(sm2[:], sm1[:], channels=128,
                                               reduce_op=bass_isa.ReduceOp.max)
                negC2 = p_l2a.tile([128, 1], dt.float32, tag="negC2",
                                   name="negC2")
                nc.vector.tensor_scalar_mul(negC2[:], sm2[:], -1.0)
                w2all = p_l2a.tile([128, NIT], dt.float32, tag="w2all",
                                   name="w2all")
                nc.scalar.activation(w2all[:], s2p[:], AF.Exp, bias=negC2[:])
                for it in range(NIT):
                    rows = slice(it * 128, (it + 1) * 128)
                    g2 = p_l2a.tile([128, PAD2], dt.float32, tag="g2", name="g2",
                                    bufs=2)
                    nc.vector.tensor_scalar_mul(g2[:, 0:NCLS], h2_sb[it][:],
                                                w2all[:, it:it + 1])
                    nc.vector.tensor_copy(g2[:, NCLS:G2C], w2all[:, it:it + 1])
                    nc.vector.memset(g2[:, G2C:PAD2], 0.0)
                    g2b = p_l2a.tile([128, PAD2], dt.bfloat16, tag="g2b",
                                     name="g2b", bufs=2)
                    nc.vector.tensor_copy(g2b[:], g2[:])
                    nc.sync.dma_start(g2_slab[rows, :], g2b[:])
                nc.gpsimd.collective_compute(
                    "AllGather", ALU.bypass, replica_groups=rg,
                    ins=[g2_slab[:]], outs=[g2_full[:]])

            # L2 adjacency matmul + final epilogue
            with (
                tc.tile_pool(name="g2t", bufs=NJT) as p_g2t,
                tc.tile_pool(name="fin", bufs=1) as p_f,
                tc.tile_pool(name="ps2", bufs=4, space="PSUM") as ps_2,
            ):
                g2v = g2_full.rearrange("(jt p) c -> jt p c", p=128)
                g2tiles = []
                for j in range(NJT):
                    gt2 = p_g2t.tile([128, PAD2], dt.bfloat16, tag="g2t",
                                     name="g2t")
                    eng = nc.sync if j % 2 == 0 else nc.scalar
                    eng.dma_start(gt2[:, 0:G2C], g2v[j, :, 0:G2C])
                    g2tiles.append(gt2)
                ps2 = [ps_2.tile([128, G2C], dt.float32, tag="ps2", name="ps2")
                       for _ in range(NIT)]
                for it in range(NIT):
                    for j in range(NJT):
                        lhs = adjt[j][:, it * 128:(it + 1) * 128]
                        nc.tensor.matmul(ps2[it][:], lhs, g2tiles[j][:, 0:G2C],
                                         start=(j == 0), stop=(j == NJT - 1))
                for it in range(NIT):
                    r2 = p_f.tile([128, 1], dt.float32, tag="r2", name="r2",
                                  bufs=2)
                    nc.vector.reciprocal(r2[:], ps2[it][:, NCLS:G2C])
                    z = p_f.tile([128, NCLS], dt.float32, tag="z2", name="z2",
                                 bufs=2)
                    nc.vector.tensor_scalar_mul(z[:], ps2[it][:, 0:NCLS], r2[:])
                    e = p_f.tile([128, NCLS], dt.float32, tag="e2", name="e2",
                                 bufs=2)
                    nc.scalar.activation(e[:], z[:], AF.Exp)
                    nc.vector.tensor_scalar(e[:], e[:], 1.0, -1.0, ALU.min,
                                            ALU.add)
                    o = p_f.tile([128, NCLS], dt.float32, tag="o2", name="o2",
                                 bufs=2)
                    nc.vector.scalar_tensor_tensor(o[:], z[:], 0.0, e[:],
                                                   ALU.max, ALU.add)
                    negm = p_f.tile([128, 1], dt.float32, tag="negm", name="negm",
                                    bufs=2)
                    nc.vector.tensor_reduce(negm[:], o[:],
                                            axis=mybir.AxisListType.X,
                                            op=ALU.max, negate=True)
                    t = p_f.tile([128, NCLS], dt.float32, tag="texp", name="texp",
                                 bufs=2)
                    nc.scalar.activation(t[:], o[:], AF.Exp, bias=negm[:])
                    ssum = p_f.tile([128, 1], dt.float32, tag="ssum", name="ssum",
                                    bufs=2)
                    nc.vector.tensor_reduce(ssum[:], t[:],
                                            axis=mybir.AxisListType.X,
                                            op=ALU.add)
                    lg = p_f.tile([128, 1], dt.float32, tag="lg", name="lg",
                                  bufs=2)
                    nc.scalar.activation(lg[:], ssum[:], AF.Ln)
                    fin = p_f.tile([128, NCLS], dt.float32, tag="fin", name="fin",
                                   bufs=2)
                    nc.vector.tensor_scalar(fin[:], o[:], negm[:], lg[:],
                                            ALU.add, ALU.subtract)
                    nc.sync.dma_start(out_d[it * 128:(it + 1) * 128, :], fin[:])

    nc.finalize()
    return nc


_CACHE = {}


def _pair(a):
    hi = a.astype(BF16)
    lo = (a - hi.astype(np.float32)).astype(BF16)
    return hi, lo


def prepare_inputs(x, adj, W_heads, a_heads, W_out, a_out):
    """Shard + lay out the full inputs for the 8 cores."""
    x2 = np.asarray(x, np.float32)[0]          # [N, F]
    adj2 = np.asarray(adj)[0]                  # [N, N] int32
    W3 = np.asarray(W_heads, np.float32).reshape(NH, F, HID)
    a3 = np.asarray(a_heads, np.float32)       # [NH, 2*HID, 1]
    Wo = np.asarray(W_out, np.float32).reshape(GH_TOT, NCLS)
    ao = np.asarray(a_out, np.float32)         # [2*NCLS, 1]

    # fold the edge-score projections into the weights:
    #   s2 = x @ (W @ a2),   s2' = xcat @ (Wo @ ao2)
    u = np.einsum("hfo,ho->hf", W3.astype(np.float64),
                  a3[:, HID:, 0].astype(np.float64)).astype(np.float32)  # [NH,F]
    u_hi, u_lo = _pair(u)
    U6 = np.zeros((F, 8), BF16)
    U3 = np.zeros((F, 8), BF16)
    for h in range(NH):
        U6[:, 2 * h] = u_hi[h]
        U6[:, 2 * h + 1] = u_lo[h]
        U3[:, h] = u_hi[h]
    u2 = (Wo.astype(np.float64) @ ao[NCLS:, 0].astype(np.float64)).astype(np.float32)
    Wo_ext = np.concatenate([Wo, u2[:, None]], axis=1)       # [GH, 257]
    Wo_b = Wo_ext.astype(BF16)
    W_b = W3.astype(BF16)
    xT = np.ascontiguousarray(x2.T)            # [F, N]
    adjb = adj2.astype(BF16)                   # exact 0/1

    in_maps = []
    for c in range(NCORES):
        sl = slice(c * SLAB, (c + 1) * SLAB)
        xh, xl = _pair(np.ascontiguousarray(xT[:, sl]))
        in_maps.append({
            "adjT": np.ascontiguousarray(adjb[sl, :].T),
            "xT_hi": xh, "xT_lo": xl,
            "U6": U6, "U3": U3,
            "W": W_b, "Wo": Wo_b,
        })
    return in_maps


def kernel(x, adj, W_heads, a_heads, W_out, a_out):
    if "nc" not in _CACHE:
        # touch the devices once so any residual bad state from a previous
        # process surfaces (and clears) before the real run
        try:
            import jax
            jax.block_until_ready(jax.numpy.zeros(8))
        except Exception:
            pass
        _CACHE["nc"] = build()
    nc = _CACHE["nc"]
    in_maps = prepare_inputs(x, adj, W_heads, a_heads, W_out, a_out)
    res = run_bass_kernel_spmd(nc, in_maps, list(range(NCORES)))
    out = np.concatenate([res.results[c]["out"] for c in range(NCORES)], axis=0)
    return out.reshape(1, N, NCLS)


# revision 15
# speedup vs baseline: 1.9343x; 1.1754x over previous
"""GAT (2-layer, 3-head) forward on 8 Trainium2 NeuronCores.

Math: with LeakyReLU slope ALPHA=1.0 the edge score e_ij = s1_i + s2_j is
linear, and s1_i cancels inside the row softmax.  The masked softmax over
j therefore reduces to column weights w_j = exp(s2_j - C) restricted to
adj, giving

    h'_i = (sum_j adj_ij * w_j * h_j) / (sum_j adj_ij * w_j)

i.e. one adjacency matmul against G = [w*h | w].  Both GAT layers take
this form (the same adjacency masks both), so the whole network is two
A-matmuls plus small projections.

Sharding: rows of h' (nodes) across 8 cores; each core holds lhsT-layout
adjacency columns A^T[:, slab] and computes its 512-row slab.  Matmuls
run in single bf16 (the tolerance is 2e-2; only the edge-score s2, which
sits in an exponent, is kept in ~fp32 via a hi/lo pair trick folded into
the input prep).  G is gathered in two chunks: [G0 | w] first so the
denominator and head-0 tiles unblock early, then [G1 | G2].
"""
import sys

sys.path.insert(0, "/opt/trn_rl_repo")

import numpy as np
import ml_dtypes

import concourse.bass as bass
import concourse.bacc as bacc
import concourse.mybir as mybir
import concourse.bass_isa as bass_isa
import concourse.tile as tile
from concourse.bass_utils import run_bass_kernel_spmd

BF16 = ml_dtypes.bfloat16

N = 4096
F = 768
HID = 768
NH = 3
NCLS = 256
NCORES = 8
SLAB = N // NCORES          # 512 rows per core
NIT = SLAB // 128           # 4 i-tiles per core
NJT = N // 128              # 32 j-tiles
NFT = F // 128              # 6 f-tiles
NCT = NH * NFT              # 18 feature col-tiles of G
G2C = NCLS + 1              # 257 = classes + s2' column (folded u2)
PAD2 = 264                  # G2 padded to 32B rows
WCOLS = 16                  # w-column slab width (3 used + pad)
GA = HID + WCOLS            # gather-A width: head0 G + w columns
GB = 2 * HID                # gather-B width: head1 + head2 G
GH_TOT = NH * HID           # 2304 xcat feature rows of Wo

AF = mybir.ActivationFunctionType
ALU = mybir.AluOpType


def _enable_ldw_opt():
    # walrus defaults to --enable-ldw-opt=false; with it off every LDWEIGHTS
    # serializes against the previous matmul (~427ns vs ~213ns per 512-col
    # matmul).  Patch the arg builder so the stationary loads pipeline.
    import concourse.bass_utils as _bu
    if getattr(_bu, "_ldw_opt_patched", False):
        return
    _orig = _bu.get_walrus_args

    def _patched(*a, **k):
        args = _orig(*a, **k)
        return [x.replace("--enable-ldw-opt=false", "--enable-ldw-opt=true")
                for x in args]

    _bu.get_walrus_args = _patched
    _bu._ldw_opt_patched = True


def build():
    dt = mybir.dt
    _enable_ldw_opt()
    nc = bacc.Bacc(num_devices=NCORES)

    adjT_d = nc.dram_tensor("adjT", [N, SLAB], dt.bfloat16, kind="ExternalInput")
    xTh_d = nc.dram_tensor("xT_hi", [F, SLAB], dt.bfloat16, kind="ExternalInput")
    xTl_d = nc.dram_tensor("xT_lo", [F, SLAB], dt.bfloat16, kind="ExternalInput")
    U6_d = nc.dram_tensor("U6", [F, 8], dt.bfloat16, kind="ExternalInput")
    U3_d = nc.dram_tensor("U3", [F, 8], dt.bfloat16, kind="ExternalInput")
    # negC[0, h] = -max_i s2_i(head h), computed exactly on the host
    negC_d = nc.dram_tensor("negC", [1, NH], dt.float32, kind="ExternalInput")
    W_d = nc.dram_tensor("W", [NH, F, HID], dt.bfloat16, kind="ExternalInput")
    Wo_d = nc.dram_tensor("Wo", [GH_TOT, G2C], dt.bfloat16, kind="ExternalInput")
    out_d = nc.dram_tensor("out", [SLAB, NCLS], dt.float32, kind="ExternalOutput")

    # DRAM scratch + collective buffers
    gsA = nc.dram_tensor("gsA", [SLAB, GA], dt.bfloat16)
    gfA = nc.dram_tensor("gfA", [N, GA], dt.bfloat16, addr_space="Shared")
    gsB = nc.dram_tensor("gsB", [SLAB, GB], dt.bfloat16)
    gfB = nc.dram_tensor("gfB", [N, GB], dt.bfloat16, addr_space="Shared")
    g2_slab = nc.dram_tensor("g2_slab", [SLAB, PAD2], dt.bfloat16)
    g2_full = nc.dram_tensor("g2_full", [N, PAD2], dt.bfloat16, addr_space="Shared")

    rg = [list(range(NCORES))]

    with tile.TileContext(nc) as tc:
      with tc.tile_pool(name="adjt", bufs=NJT) as p_adjt:
        # ---------------- phase 1: s2, w, h=x@W, G build + gathers ----------
        with (
            tc.tile_pool(name="xw", bufs=1) as p_xw,
            tc.tile_pool(name="small", bufs=1) as p_sm,
            tc.tile_pool(name="gtmp", bufs=1) as p_gt,
        ):
            # x tiles first (s2 needs all of them), W next (x@W starts as
            # soon as head-0 W lands), adjacency last.
            xhi, xlo = [], []
            xTh_t = xTh_d.rearrange("(ft p) i -> ft p i", p=128)
            xTl_t = xTl_d.rearrange("(ft p) i -> ft p i", p=128)
            for ft in range(NFT):
                t = p_xw.tile([128, SLAB], dt.bfloat16, tag="x", name="x", bufs=12)
                nc.sync.dma_start(t[:], xTh_t[ft])
                xhi.append(t)
                t = p_xw.tile([128, SLAB], dt.bfloat16, tag="x", name="x", bufs=12)
                nc.scalar.dma_start(t[:], xTl_t[ft])
                xlo.append(t)
            u6 = p_sm.tile([128, NFT, 8], dt.bfloat16, tag="u6", name="u6")
            nc.sync.dma_start(u6[:], U6_d.rearrange("(ft p) c -> p ft c", p=128))
            u3 = p_sm.tile([128, NFT, 8], dt.bfloat16, tag="u3", name="u3")
            nc.scalar.dma_start(u3[:], U3_d.rearrange("(ft p) c -> p ft c", p=128))
            negC = p_sm.tile([1, NH], dt.float32, tag="negC", name="negC")
            nc.gpsimd.dma_start(negC[:], negC_d[:])
            negCbc = p_sm.tile([128, NH], dt.float32, tag="negCbc", name="negCbc")
            nc.gpsimd.partition_broadcast(negCbc[:], negC[:], channels=128)

            wt_sb = [[None] * NFT for _ in range(NH)]
            W_t = W_d.rearrange("h (ft p) o -> h ft p o", p=128)
            for h in range(NH):
                for ft in range(NFT):
                    t = p_xw.tile([128, HID], dt.bfloat16, tag="w", name="w",
                                  bufs=NH * NFT)
                    eng = nc.sync if (h * NFT + ft) % 2 == 0 else nc.scalar
                    eng.dma_start(t[:], W_t[h, ft])
                    wt_sb[h][ft] = t
            # adjacency on the scalar queue only, so the sync queue is free
            # for the G staging writes that gate the gathers
            adjt = []
            adjT_t = adjT_d.rearrange("(jt p) i -> jt p i", p=128)
            for j in range(NJT):
                t = p_adjt.tile([128, SLAB], dt.bfloat16, tag="adjt", name="adjt")
                nc.scalar.dma_start(t[:], adjT_t[j])
                adjt.append(t)

            # s2 = x @ u (tiny matmuls, hi/lo pair precision), slab max,
            # tiny AllGather.
            s2_sb = []
            for h in range(NH):
                s2_sb.append(p_sm.tile([128, NIT], dt.float32, tag="s2",
                                       name="s2", bufs=NH))
            with tc.tile_pool(name="psS", bufs=2, space="PSUM") as ps_s:
                for it in range(NIT):
                    p6 = ps_s.tile([128, 8], dt.float32, tag="p6", name="p6", bufs=2)
                    p3 = ps_s.tile([128, 8], dt.float32, tag="p3", name="p3", bufs=2)
                    for ft in range(NFT):
                        xh = xhi[ft][:, it * 128:(it + 1) * 128]
                        xl = xlo[ft][:, it * 128:(it + 1) * 128]
                        nc.tensor.matmul(p6[:], xh, u6[:, ft, :],
                                         start=(ft == 0), stop=(ft == NFT - 1))
                        nc.tensor.matmul(p3[:], xl, u3[:, ft, :],
                                         start=(ft == 0), stop=(ft == NFT - 1))
                    t6 = p_sm.tile([128, 8], dt.float32, tag="t6",
                                   name="t6", bufs=2)
                    nc.vector.tensor_copy(t6[:], p6[:])
                    tsum = p_sm.tile([128, NH], dt.float32, tag="tsum",
                                     name="tsum", bufs=2)
                    nc.vector.tensor_tensor(tsum[:], t6[:, 0:2 * NH:2],
                                            t6[:, 1:2 * NH:2], ALU.add)
                    for h in range(NH):
                        nc.vector.tensor_tensor(s2_sb[h][:, it:it + 1],
                                                tsum[:, h:h + 1], p3[:, h:h + 1],
                                                ALU.add)

            # w = exp(s2 - C) with the host-computed C — no collective needed
            w_sb = []
            for h in range(NH):
                w = p_sm.tile([128, NIT], dt.float32, tag="wexp", name="wexp",
                              bufs=NH)
                nc.scalar.activation(w[:], s2_sb[h][:], AF.Exp,
                                     bias=negCbc[:, h:h + 1])
                w_sb.append(w)
            wb3 = p_sm.tile([128, NH, NIT], dt.bfloat16, tag="wb3", name="wb3")
            for h in range(NH):
                nc.vector.tensor_copy(wb3[:, h, :], w_sb[h][:])
            for it in range(NIT):
                rows = slice(it * 128, (it + 1) * 128)
                wt = p_sm.tile([128, WCOLS], dt.bfloat16, tag="wt", name="wt",
                               bufs=2)
                nc.vector.memset(wt[:], 0.0)
                nc.vector.tensor_copy(wt[:, 0:NH], wb3[:, :, it])
                nc.gpsimd.dma_start(gsA[rows, HID:GA], wt[:])

            # h = x@W, scaled to G as each tile drains; gather A as soon as
            # head 0 is done, so the denominator and head-0 tiles unblock.
            with tc.tile_pool(name="psA", bufs=2, space="PSUM") as ps_a:
                for h in range(NH):
                    for it in range(NIT):
                        ps = ps_a.tile([128, HID], dt.float32, tag="psA", name="psA")
                        for ft in range(NFT):
                            xh = xhi[ft][:, it * 128:(it + 1) * 128]
                            w = wt_sb[h][ft]
                            nc.tensor.matmul(ps[:, 0:512], xh, w[:, 0:512],
                                             start=(ft == 0), stop=(ft == NFT - 1))
                            nc.tensor.matmul(ps[:, 512:HID], xh, w[:, 512:HID],
                                             start=(ft == 0), stop=(ft == NFT - 1))
                        g = p_gt.tile([128, HID], dt.bfloat16, tag="g0",
                                      name="g0", bufs=4)
                        nc.vector.tensor_scalar_mul(g[:], ps[:],
                                                    w_sb[h][:, it:it + 1])
                        rows = slice(it * 128, (it + 1) * 128)
                        if h == 0:
                            nc.sync.dma_start(gsA[rows, 0:HID], g[:])
                        else:
                            eng = nc.sync if h == 1 else nc.scalar
                            eng.dma_start(gsB[rows, (h - 1) * HID:h * HID], g[:])
                    if h == 0:
                        nc.gpsimd.collective_compute(
                            "AllGather", ALU.bypass, replica_groups=rg,
                            ins=[gsA[:]], outs=[gfA[:]])
                nc.gpsimd.collective_compute(
                    "AllGather", ALU.bypass, replica_groups=rg,
                    ins=[gsB[:]], outs=[gfB[:]])

        # ---------------- L1 adjacency matmul + epilogue + layer 2 ----------
        with tc.tile_pool(name="xct", bufs=1) as p_xct:
            with (
                tc.tile_pool(name="gst", bufs=8) as p_gst,
                tc.tile_pool(name="etmp", bufs=1) as p_et,
                tc.tile_pool(name="wo", bufs=1) as p_wo,
                tc.tile_pool(name="l2a", bufs=1) as p_l2a,
                tc.tile_pool(name="ps1", bufs=4, space="PSUM") as ps_1,
                tc.tile_pool(name="psh2", bufs=4, space="PSUM") as ps_h2,
            ):
                # Wo loads early; they only feed the inline h2 matmuls
                wo_sb = []
                Wo_t = Wo_d.rearrange("(ot p) c -> ot p c", p=128)
                for ot in range(NCT):
                    t = p_wo.tile([128, G2C], dt.bfloat16, tag="wo", name="wo",
                                  bufs=NCT)
                    eng = nc.sync if ot % 2 == 0 else nc.scalar
                    eng.dma_start(t[:], Wo_t[ot])
                    wo_sb.append(t)

                # denominator col-tile first: den_k = A @ w_k
                gwv = gfA.rearrange("(jt p) c -> p jt c", p=128)
                gwt = p_gst.tile([128, NJT, WCOLS], dt.bfloat16, tag="gwt",
                                 name="gwt", bufs=1)
                nc.sync.dma_start(gwt[:], gwv[:, :, HID:GA])
                psd = ps_1.tile([128, SLAB], dt.float32, tag="ps1", name="ps1")
                for j in range(NJT):
                    nc.tensor.matmul(psd[0:NH, :], gwt[:, j, 0:NH], adjt[j][:],
                                     start=(j == 0), stop=(j == NJT - 1))
                recip3 = p_et.tile([NH, SLAB], dt.float32, tag="recip3",
                                   name="recip3")
                nc.vector.reciprocal(recip3[:], psd[0:NH, :])
                rbc = []
                for h in range(NH):
                    rrow = p_et.tile([1, SLAB], dt.float32, tag="rrow",
                                     name="rrow", bufs=2)
                    nc.sync.dma_start(rrow[:], recip3[h:h + 1, :])
                    rb = p_et.tile([128, SLAB], dt.float32, tag="rbc",
                                   name="rbc", bufs=NH)
                    nc.gpsimd.partition_broadcast(rb[:], rrow[:], channels=128)
                    rbc.append(rb)

                # feature col-tiles, head-major; epilogue + h2 inline per ct
                gvA = gfA.rearrange("(jb q p) c -> jb p q c", q=4, p=128)
                gvB = gfB.rearrange("(jb q p) c -> jb p q c", q=4, p=128)
                ps2l = [ps_h2.tile([128, G2C], dt.float32, tag="psh2",
                                   name="psh2") for _ in range(NIT)]
                for ct in range(NCT):
                    h = ct // NFT
                    lct = ct % NFT
                    ps = ps_1.tile([128, SLAB], dt.float32, tag="ps1", name="ps1")
                    for jb in range(NJT // 4):
                        gt = p_gst.tile([128, 4, 128], dt.bfloat16, tag="gst",
                                        name="gst")
                        eng = nc.sync if jb % 2 == 0 else nc.scalar
                        if h == 0:
                            eng.dma_start(gt[:], gvA[jb, :, :,
                                                     lct * 128:(lct + 1) * 128])
                        else:
                            c0 = (h - 1) * HID + lct * 128
                            eng.dma_start(gt[:], gvB[jb, :, :, c0:c0 + 128])
                        for q in range(4):
                            j = jb * 4 + q
                            nc.tensor.matmul(ps[:], gt[:, q, :], adjt[j][:],
                                             start=(j == 0), stop=(j == NJT - 1))
                    # xcatT tile = elu(numT / den), bf16
                    z = p_et.tile([128, SLAB], dt.float32, tag="z", name="z",
                                  bufs=2)
                    nc.vector.tensor_tensor(z[:], ps[:], rbc[h][:], ALU.mult)
                    e = p_et.tile([128, SLAB], dt.float32, tag="e", name="e",
                                  bufs=2)
                    nc.scalar.activation(e[:], z[:], AF.Exp)
                    nc.vector.tensor_scalar(e[:], e[:], 1.0, -1.0, ALU.min,
                                            ALU.add)
                    xc = p_xct.tile([128, SLAB], dt.bfloat16, tag="xcp",
                                    name="xcp", bufs=NCT)
                    nc.vector.scalar_tensor_tensor(xc[:], z[:], 0.0, e[:],
                                                   ALU.max, ALU.add)
                    # layer 2 accumulation: h2 += xcat_ct @ Wo_ct
                    for it in range(NIT):
                        nc.tensor.matmul(ps2l[it][:],
                                         xc[:, it * 128:(it + 1) * 128],
                                         wo_sb[ct][:],
                                         start=(ct == 0), stop=(ct == NCT - 1))

                # layer-2 weights w2 = exp(s2') with NO max subtraction:
                # s2' stays well under fp32/bf16 exp range and the common
                # scale cancels exactly in num/den.
                for it in range(NIT):
                    rows = slice(it * 128, (it + 1) * 128)
                    w2 = p_l2a.tile([128, 1], dt.float32, tag="w2", name="w2",
                                    bufs=2)
                    nc.scalar.activation(w2[:], ps2l[it][:, NCLS:G2C], AF.Exp)
                    g2 = p_l2a.tile([128, PAD2], dt.float32, tag="g2", name="g2",
                                    bufs=2)
                    nc.vector.tensor_scalar_mul(g2[:, 0:NCLS],
                                                ps2l[it][:, 0:NCLS], w2[:])
                    nc.vector.tensor_copy(g2[:, NCLS:G2C], w2[:])
                    nc.vector.memset(g2[:, G2C:PAD2], 0.0)
                    g2b = p_l2a.tile([128, PAD2], dt.bfloat16, tag="g2b",
                                     name="g2b", bufs=2)
                    nc.vector.tensor_copy(g2b[:], g2[:])
                    nc.sync.dma_start(g2_slab[rows, :], g2b[:])
                nc.gpsimd.collective_compute(
                    "AllGather", ALU.bypass, replica_groups=rg,
                    ins=[g2_slab[:]], outs=[g2_full[:]])

            # L2 adjacency matmul + final epilogue
            with (
                tc.tile_pool(name="g2t", bufs=NJT) as p_g2t,
                tc.tile_pool(name="fin", bufs=1) as p_f,
                tc.tile_pool(name="ps2", bufs=4, space="PSUM") as ps_2,
            ):
                g2v = g2_full.rearrange("(jt p) c -> jt p c", p=128)
                g2tiles = []
                for j in range(NJT):
                    gt2 = p_g2t.tile([128, PAD2], dt.bfloat16, tag="g2t",
                                     name="g2t")
                    eng = nc.sync if j % 2 == 0 else nc.scalar
                    eng.dma_start(gt2[:], g2v[j])
                    g2tiles.append(gt2)
                ps2 = [ps_2.tile([128, G2C], dt.float32, tag="ps2", name="ps2")
                       for _ in range(NIT)]
                for it in range(NIT):
                    for j in range(NJT):
                        lhs = adjt[j][:, it * 128:(it + 1) * 128]
                        nc.tensor.matmul(ps2[it][:], lhs, g2tiles[j][:, 0:G2C],
                                         start=(j == 0), stop=(j == NJT - 1))
                for it in range(NIT):
                    r2 = p_f.tile([128, 1], dt.float32, tag="r2", name="r2",
                                  bufs=2)
                    nc.vector.reciprocal(r2[:], ps2[it][:, NCLS:G2C])
                    z = p_f.tile([128, NCLS], dt.float32, tag="z2", name="z2",
                                 bufs=2)
                    nc.vector.tensor_scalar_mul(z[:], ps2[it][:, 0:NCLS], r2[:])
                    e = p_f.tile([128, NCLS], dt.float32, tag="e2", name="e2",
                                 bufs=2)
                    nc.scalar.activation(e[:], z[:], AF.Exp)
                    nc.vector.tensor_scalar(e[:], e[:], 1.0, -1.0, ALU.min,
                                            ALU.add)
                    o = p_f.tile([128, NCLS], dt.float32, tag="o2", name="o2",
                                 bufs=2)
                    nc.vector.scalar_tensor_tensor(o[:], z[:], 0.0, e[:],
                                                   ALU.max, ALU.add)
                    negm = p_f.tile([128, 1], dt.float32, tag="negm", name="negm",
                                    bufs=2)
                    nc.vector.tensor_reduce(negm[:], o[:],
                                            axis=mybir.AxisListType.X,
                                            op=ALU.max, negate=True)
                    t = p_f.tile([128, NCLS], dt.float32, tag="texp", name="texp",
                                 bufs=2)
                    nc.scalar.activation(t[:], o[:], AF.Exp, bias=negm[:])
                    ssum = p_f.tile([128, 1], dt.float32, tag="ssum", name="ssum",
                                    bufs=2)
                    nc.vector.tensor_reduce(ssum[:], t[:],
                                            axis=mybir.AxisListType.X,
                                            op=ALU.add)
                    lg = p_f.tile([128, 1], dt.float32, tag="lg", name="lg",
                                  bufs=2)
                    nc.scalar.activation(lg[:], ssum[:], AF.Ln)
                    fin = p_f.tile([128, NCLS], dt.float32, tag="fin", name="fin",
                                   bufs=2)
                    nc.vector.tensor_scalar(fin[:], o[:], negm[:], lg[:],
                                            ALU.add, ALU.subtract)
                    nc.sync.dma_start(out_d[it * 128:(it + 1) * 128, :], fin[:])

    nc.finalize()
    return nc


_CACHE = {}


def _pair(a):
    hi = a.astype(BF16)
    lo = (a - hi.astype(np.float32)).astype(BF16)
    return hi, lo


def prepare_inputs(x, adj, W_heads, a_heads, W_out, a_out):
    """Shard + lay out the full inputs for the 8 cores."""
    x2 = np.asarray(x, np.float32)[0]          # [N, F]
    adj2 = np.asarray(adj)[0]                  # [N, N] int32
    W3 = np.asarray(W_heads, np.float32).reshape(NH, F, HID)
    a3 = np.asarray(a_heads, np.float32)       # [NH, 2*HID, 1]
    Wo = np.asarray(W_out, np.float32).reshape(GH_TOT, NCLS)
    ao = np.asarray(a_out, np.float32)         # [2*NCLS, 1]

    # fold the edge-score projections into the weights:
    #   s2 = x @ (W @ a2),   s2' = xcat @ (Wo @ ao2)
    u = np.einsum("hfo,ho->hf", W3.astype(np.float64),
                  a3[:, HID:, 0].astype(np.float64)).astype(np.float32)  # [NH,F]
    u_hi, u_lo = _pair(u)
    U6 = np.zeros((F, 8), BF16)
    U3 = np.zeros((F, 8), BF16)
    for h in range(NH):
        U6[:, 2 * h] = u_hi[h]
        U6[:, 2 * h + 1] = u_lo[h]
        U3[:, h] = u_hi[h]
    u2 = (Wo.astype(np.float64) @ ao[NCLS:, 0].astype(np.float64)).astype(np.float32)
    Wo_ext = np.concatenate([Wo, u2[:, None]], axis=1)       # [GH, 257]
    Wo_b = Wo_ext.astype(BF16)
    W_b = W3.astype(BF16)
    xT = np.ascontiguousarray(x2.T)            # [F, N]
    adjb = adj2.astype(BF16)                   # exact 0/1

    # exact per-head max of s2 = x @ u, folded on the host so the device
    # needs no max-reduction collective.  Mirror the device arithmetic
    # (bf16 hi/lo pair dot products accumulated in fp32).
    xh_f, xl_f = _pair(x2)
    s2 = (xh_f.astype(np.float32) @ u_hi.T.astype(np.float32)
          + xh_f.astype(np.float32) @ u_lo.T.astype(np.float32)
          + xl_f.astype(np.float32) @ u_hi.T.astype(np.float32))  # [N, NH]
    negC = -s2.max(axis=0, keepdims=True).astype(np.float32)      # [1, NH]

    in_maps = []
    for c in range(NCORES):
        sl = slice(c * SLAB, (c + 1) * SLAB)
        xh, xl = _pair(np.ascontiguousarray(xT[:, sl]))
        in_maps.append({
            "adjT": np.ascontiguousarray(adjb[sl, :].T),
            "xT_hi": xh, "xT_lo": xl,
            "U6": U6, "U3": U3, "negC": negC,
            "W": W_b, "Wo": Wo_b,
        })
    return in_maps


def kernel(x, adj, W_heads, a_heads, W_out, a_out):
    if "nc" not in _CACHE:
        # touch the devices once so any residual bad state from a previous
        # process surfaces (and clears) before the real run
        try:
            import jax
            jax.block_until_ready(jax.numpy.zeros(8))
        except Exception:
            pass
        _CACHE["nc"] = build()
    nc = _CACHE["nc"]
    in_maps = prepare_inputs(x, adj, W_heads, a_heads, W_out, a_out)
    res = run_bass_kernel_spmd(nc, in_maps, list(range(NCORES)))
    out = np.concatenate([res.results[c]["out"] for c in range(NCORES)], axis=0)
    return out.reshape(1, N, NCLS)


# revision 28
# speedup vs baseline: 2.0162x; 1.0423x over previous
"""GAT (2-layer, 3-head) forward on 8 Trainium2 NeuronCores.

Math: with LeakyReLU slope ALPHA=1.0 the edge score e_ij = s1_i + s2_j is
linear, and s1_i cancels inside the row softmax.  The masked softmax over
j therefore reduces to column weights w_j = exp(s2_j - C) restricted to
adj, giving

    h'_i = (sum_j adj_ij * w_j * h_j) / (sum_j adj_ij * w_j)

i.e. one adjacency matmul against G = [w*h | w].  Both GAT layers take
this form (the same adjacency masks both), so the whole network is two
A-matmuls plus small projections.

Sharding: rows of h' (nodes) across 8 cores; each core holds lhsT-layout
adjacency columns A^T[:, slab] and computes its 512-row slab.  Matmuls
run in single bf16 (the tolerance is 2e-2; only the edge-score s2, which
sits in an exponent, is kept in ~fp32 via a hi/lo pair trick folded into
the input prep).  G is gathered in two chunks: [G0 | w] first so the
denominator and head-0 tiles unblock early, then [G1 | G2].
"""
import sys

sys.path.insert(0, "/opt/trn_rl_repo")

import numpy as np
import ml_dtypes

import concourse.bass as bass
import concourse.bacc as bacc
import concourse.mybir as mybir
import concourse.bass_isa as bass_isa
import concourse.tile as tile
from concourse.bass_utils import run_bass_kernel_spmd

BF16 = ml_dtypes.bfloat16

N = 4096
F = 768
HID = 768
NH = 3
NCLS = 256
NCORES = 8
SLAB = N // NCORES          # 512 rows per core
NIT = SLAB // 128           # 4 i-tiles per core
NJT = N // 128              # 32 j-tiles
NFT = F // 128              # 6 f-tiles
NCT = NH * NFT              # 18 feature col-tiles of G
G2C = NCLS + 1              # 257 = classes + s2' column (folded u2)
PAD2 = 264                  # G2 padded to 32B rows
WCOLS = 16                  # w-column slab width (3 used + pad)
GA = WCOLS + HID            # gather-A width: [w cols | head0 G]
GB = 2 * HID                # gather-B width: head1 + head2 G
GH_TOT = NH * HID           # 2304 xcat feature rows of Wo

AF = mybir.ActivationFunctionType
ALU = mybir.AluOpType


def _enable_ldw_opt():
    # walrus defaults to --enable-ldw-opt=false; with it off every LDWEIGHTS
    # serializes against the previous matmul (~427ns vs ~213ns per 512-col
    # matmul).  Patch the arg builder so the stationary loads pipeline.
    import concourse.bass_utils as _bu
    if getattr(_bu, "_ldw_opt_patched", False):
        return
    _orig = _bu.get_walrus_args

    def _patched(*a, **k):
        args = _orig(*a, **k)
        return [x.replace("--enable-ldw-opt=false", "--enable-ldw-opt=true")
                for x in args]

    _bu.get_walrus_args = _patched
    _bu._ldw_opt_patched = True


def build():
    dt = mybir.dt
    _enable_ldw_opt()
    nc = bacc.Bacc(num_devices=NCORES)

    adjT_d = nc.dram_tensor("adjT", [N, SLAB], dt.bfloat16, kind="ExternalInput")
    xTh_d = nc.dram_tensor("xT_hi", [F, SLAB], dt.bfloat16, kind="ExternalInput")
    xTl_d = nc.dram_tensor("xT_lo", [F, SLAB], dt.bfloat16, kind="ExternalInput")
    U6_d = nc.dram_tensor("U6", [F, 8], dt.bfloat16, kind="ExternalInput")
    U3_d = nc.dram_tensor("U3", [F, 8], dt.bfloat16, kind="ExternalInput")
    # negC[0, h] = -max_i s2_i(head h), computed exactly on the host
    negC_d = nc.dram_tensor("negC", [1, NH], dt.float32, kind="ExternalInput")
    W_d = nc.dram_tensor("W", [NH, F, HID], dt.bfloat16, kind="ExternalInput")
    Wo_d = nc.dram_tensor("Wo", [GH_TOT, G2C], dt.bfloat16, kind="ExternalInput")
    out_d = nc.dram_tensor("out", [SLAB, NCLS], dt.float32, kind="ExternalOutput")

    # DRAM scratch + collective buffers
    gsA = nc.dram_tensor("gsA", [SLAB, GA], dt.bfloat16)
    gfA = nc.dram_tensor("gfA", [N, GA], dt.bfloat16, addr_space="Shared")
    gsB = nc.dram_tensor("gsB", [SLAB, GB], dt.bfloat16)
    gfB = nc.dram_tensor("gfB", [N, GB], dt.bfloat16, addr_space="Shared")
    # g2 gathered in two half-slab chunks so the L2 matmul can start on the
    # first half while the second is in flight
    g2_slab = [nc.dram_tensor(f"g2_slab{k}", [SLAB // 2, PAD2], dt.bfloat16)
               for k in range(2)]
    g2_full = [nc.dram_tensor(f"g2_full{k}", [N // 2, PAD2], dt.bfloat16,
                              addr_space="Shared") for k in range(2)]

    rg = [list(range(NCORES))]

    with tile.TileContext(nc) as tc:
      with tc.tile_pool(name="adjt", bufs=NJT) as p_adjt:
        # ---------------- phase 1: s2, w, h=x@W, G build + gathers ----------
        with (
            tc.tile_pool(name="xw", bufs=1) as p_xw,
            tc.tile_pool(name="small", bufs=1) as p_sm,
            tc.tile_pool(name="gtmp", bufs=1) as p_gt,
        ):
            # Batched input loads: one big DMA per tensor (chunked transfers
            # serialize at ~650ns per 128KB, so 70 small DMAs would cost
            # ~45us of serial load time).  x + head-0 W first: they gate
            # s2 and the first x@W matmuls.
            xh_all = p_xw.tile([128, NFT, SLAB], dt.bfloat16, tag="xh", name="xh")
            nc.sync.dma_start(xh_all[:],
                              xTh_d.rearrange("(ft p) i -> p ft i", p=128))
            xl_all = p_xw.tile([128, NFT, SLAB], dt.bfloat16, tag="xl", name="xl")
            nc.scalar.dma_start(xl_all[:],
                                xTl_d.rearrange("(ft p) i -> p ft i", p=128))
            def xhi(ft, c0, c1):
                return xh_all[:, ft, c0:c1]

            def xlo(ft, c0, c1):
                return xl_all[:, ft, c0:c1]
            u6 = p_sm.tile([128, NFT, 8], dt.bfloat16, tag="u6", name="u6")
            nc.gpsimd.dma_start(u6[:], U6_d.rearrange("(ft p) c -> p ft c", p=128))
            u3 = p_sm.tile([128, NFT, 8], dt.bfloat16, tag="u3", name="u3")
            nc.gpsimd.dma_start(u3[:], U3_d.rearrange("(ft p) c -> p ft c", p=128))
            negC = p_sm.tile([1, NH], dt.float32, tag="negC", name="negC")
            nc.gpsimd.dma_start(negC[:], negC_d[:])
            negCbc = p_sm.tile([128, NH], dt.float32, tag="negCbc", name="negCbc")
            nc.gpsimd.partition_broadcast(negCbc[:], negC[:], channels=128)

            W_t = W_d.rearrange("h (ft p) o -> p h ft o", p=128)
            w0_all = p_xw.tile([128, NFT, HID], dt.bfloat16, tag="w0", name="w0")
            nc.sync.dma_start(w0_all[:], W_t[:, 0])
            w12_all = p_xw.tile([128, 2, NFT, HID], dt.bfloat16, tag="w12",
                                name="w12")
            nc.scalar.dma_start(w12_all[:], W_t[:, 1:3])

            def wsl(h, ft, c0, c1):
                if h == 0:
                    return w0_all[:, ft, c0:c1]
                return w12_all[:, h - 1, ft, c0:c1]

            adjt_all = []
            adjT_t = adjT_d.rearrange("(half jh p) i -> half p jh i",
                                      half=2, p=128)
            for half in range(2):
                t = p_adjt.tile([128, NJT // 2, SLAB], dt.bfloat16, tag="adjt",
                                name="adjt", bufs=2)
                eng = nc.sync if half == 0 else nc.scalar
                eng.dma_start(t[:], adjT_t[half])
                adjt_all.append(t)

            def adjs(j, c0=0, c1=SLAB):
                return adjt_all[j // (NJT // 2)][:, j % (NJT // 2), c0:c1]

            # s2 = x @ u (tiny matmuls, hi/lo pair precision), slab max,
            # tiny AllGather.
            s2_sb = []
            for h in range(NH):
                s2_sb.append(p_sm.tile([128, NIT], dt.float32, tag="s2",
                                       name="s2", bufs=NH))
            with tc.tile_pool(name="psS", bufs=2, space="PSUM") as ps_s:
                for it in range(NIT):
                    p6 = ps_s.tile([128, 8], dt.float32, tag="p6", name="p6", bufs=2)
                    p3 = ps_s.tile([128, 8], dt.float32, tag="p3", name="p3", bufs=2)
                    for ft in range(NFT):
                        xh = xhi(ft, it * 128, (it + 1) * 128)
                        xl = xlo(ft, it * 128, (it + 1) * 128)
                        nc.tensor.matmul(p6[:], xh, u6[:, ft, :],
                                         start=(ft == 0), stop=(ft == NFT - 1))
                        nc.tensor.matmul(p3[:], xl, u3[:, ft, :],
                                         start=(ft == 0), stop=(ft == NFT - 1))
                    t6 = p_sm.tile([128, 8], dt.float32, tag="t6",
                                   name="t6", bufs=2)
                    nc.vector.tensor_copy(t6[:], p6[:])
                    tsum = p_sm.tile([128, NH], dt.float32, tag="tsum",
                                     name="tsum", bufs=2)
                    nc.vector.tensor_tensor(tsum[:], t6[:, 0:2 * NH:2],
                                            t6[:, 1:2 * NH:2], ALU.add)
                    for h in range(NH):
                        nc.vector.tensor_tensor(s2_sb[h][:, it:it + 1],
                                                tsum[:, h:h + 1], p3[:, h:h + 1],
                                                ALU.add)

            # w = exp(s2 - C) with the host-computed C — no collective needed
            w_sb = []
            for h in range(NH):
                w = p_sm.tile([128, NIT], dt.float32, tag="wexp", name="wexp",
                              bufs=NH)
                nc.scalar.activation(w[:], s2_sb[h][:], AF.Exp,
                                     bias=negCbc[:, h:h + 1])
                w_sb.append(w)
            wb3 = p_sm.tile([128, NH, NIT], dt.bfloat16, tag="wb3", name="wb3")
            for h in range(NH):
                nc.vector.tensor_copy(wb3[:, h, :], w_sb[h][:])
            for it in range(NIT):
                rows = slice(it * 128, (it + 1) * 128)
                wt = p_sm.tile([128, WCOLS], dt.bfloat16, tag="wt", name="wt",
                               bufs=2)
                nc.vector.memset(wt[:], 0.0)
                nc.vector.tensor_copy(wt[:, 0:NH], wb3[:, :, it])
                nc.gpsimd.dma_start(gsA[rows, 0:WCOLS], wt[:])

            # h = x@W, scaled to G as each tile drains; gather A as soon as
            # head 0 is done, so the denominator and head-0 tiles unblock.
            with tc.tile_pool(name="psA", bufs=2, space="PSUM") as ps_a:
                for h in range(NH):
                    for it in range(NIT):
                        ps = ps_a.tile([128, HID], dt.float32, tag="psA", name="psA")
                        for ft in range(NFT):
                            xh = xhi(ft, it * 128, (it + 1) * 128)
                            nc.tensor.matmul(ps[:, 0:512], xh, wsl(h, ft, 0, 512),
                                             start=(ft == 0), stop=(ft == NFT - 1))
                            nc.tensor.matmul(ps[:, 512:HID], xh,
                                             wsl(h, ft, 512, HID),
                                             start=(ft == 0), stop=(ft == NFT - 1))
                        g = p_gt.tile([128, HID], dt.bfloat16, tag="g0",
                                      name="g0", bufs=4)
                        nc.vector.tensor_scalar_mul(g[:], ps[:],
                                                    w_sb[h][:, it:it + 1])
                        rows = slice(it * 128, (it + 1) * 128)
                        if h == 0:
                            nc.sync.dma_start(gsA[rows, WCOLS:GA], g[:])
                        else:
                            eng = nc.sync if h == 1 else nc.scalar
                            eng.dma_start(gsB[rows, (h - 1) * HID:h * HID], g[:])
                    if h == 0:
                        nc.gpsimd.collective_compute(
                            "AllGather", ALU.bypass, replica_groups=rg,
                            ins=[gsA[:]], outs=[gfA[:]])
                nc.gpsimd.collective_compute(
                    "AllGather", ALU.bypass, replica_groups=rg,
                    ins=[gsB[:]], outs=[gfB[:]])

        # ---------------- L1 adjacency matmul + epilogue + layer 2 ----------
        with tc.tile_pool(name="xct", bufs=1) as p_xct:
            with (
                tc.tile_pool(name="gst", bufs=8) as p_gst,
                tc.tile_pool(name="etmp", bufs=1) as p_et,
                tc.tile_pool(name="wo", bufs=1) as p_wo,
                tc.tile_pool(name="l2a", bufs=1) as p_l2a,
                tc.tile_pool(name="ps1", bufs=4, space="PSUM") as ps_1,
                tc.tile_pool(name="psh2", bufs=4, space="PSUM") as ps_h2,
            ):
                # Wo loads early; they only feed the inline h2 matmuls
                wo_sb = []
                Wo_t = Wo_d.rearrange("(ot p) c -> ot p c", p=128)
                for ot in range(NCT):
                    t = p_wo.tile([128, G2C], dt.bfloat16, tag="wo", name="wo",
                                  bufs=NCT)
                    eng = nc.sync if ot % 2 == 0 else nc.scalar
                    eng.dma_start(t[:], Wo_t[ot])
                    wo_sb.append(t)

                # feature col-tiles, head-major; epilogue + h2 inline per ct.
                # ct 0 also carries the w columns (first WCOLS of gfA), so the
                # denominator matmuls ride its tile loads — no separate
                # strided gather of w.
                gvA = gfA.rearrange("(jb q p) c -> jb p q c", q=4, p=128)
                gvB = gfB.rearrange("(jb q p) c -> jb p q c", q=4, p=128)
                ps2l = [ps_h2.tile([128, G2C], dt.float32, tag="psh2",
                                   name="psh2") for _ in range(NIT)]
                rbc = [None] * NH
                psd = ps_1.tile([128, SLAB], dt.float32, tag="psd", name="psd",
                                bufs=1)
                for ct in range(NCT):
                    h = ct // NFT
                    lct = ct % NFT
                    ps = ps_1.tile([128, SLAB], dt.float32, tag="ps1", name="ps1",
                                   bufs=3)
                    for jb in range(NJT // 4):
                        if ct == 0:
                            gt = p_gst.tile([128, 4, WCOLS + 128], dt.bfloat16,
                                            tag="gst0", name="gst0", bufs=8)
                            eng = nc.sync if jb % 2 == 0 else nc.scalar
                            eng.dma_start(gt[:], gvA[jb, :, :, 0:WCOLS + 128])
                            goff = WCOLS
                        elif h == 0:
                            gt = p_gst.tile([128, 4, 128], dt.bfloat16,
                                            tag="gst", name="gst")
                            eng = nc.sync if jb % 2 == 0 else nc.scalar
                            eng.dma_start(gt[:], gvA[jb, :, :,
                                                     WCOLS + lct * 128:
                                                     WCOLS + (lct + 1) * 128])
                            goff = 0
                        else:
                            gt = p_gst.tile([128, 4, 128], dt.bfloat16,
                                            tag="gst", name="gst")
                            eng = nc.sync if jb % 2 == 0 else nc.scalar
                            c0 = (h - 1) * HID + lct * 128
                            eng.dma_start(gt[:], gvB[jb, :, :, c0:c0 + 128])
                            goff = 0
                        for q in range(4):
                            j = jb * 4 + q
                            if ct == 0:
                                nc.tensor.matmul(psd[0:NH, :], gt[:, q, 0:NH],
                                                 adjs(j), start=(j == 0),
                                                 stop=(j == NJT - 1))
                            nc.tensor.matmul(ps[:], gt[:, q, goff:goff + 128],
                                             adjs(j), start=(j == 0),
                                             stop=(j == NJT - 1))
                    if ct == 0:
                        recip3 = p_et.tile([NH, SLAB], dt.float32, tag="recip3",
                                           name="recip3")
                        nc.vector.reciprocal(recip3[:], psd[0:NH, :])
                        for hh in range(NH):
                            rrow = p_et.tile([1, SLAB], dt.float32, tag="rrow",
                                             name="rrow", bufs=2)
                            nc.sync.dma_start(rrow[:], recip3[hh:hh + 1, :])
                            rb = p_et.tile([128, SLAB], dt.float32, tag="rbc",
                                           name="rbc", bufs=NH)
                            nc.gpsimd.partition_broadcast(rb[:], rrow[:],
                                                          channels=128)
                            rbc[hh] = rb
                    # xcatT tile = elu(numT / den), bf16
                    z = p_et.tile([128, SLAB], dt.float32, tag="z", name="z",
                                  bufs=2)
                    nc.vector.tensor_tensor(z[:], ps[:], rbc[h][:], ALU.mult)
                    e = p_et.tile([128, SLAB], dt.float32, tag="e", name="e",
                                  bufs=2)
                    nc.scalar.activation(e[:], z[:], AF.Exp)
                    nc.vector.tensor_scalar(e[:], e[:], 1.0, -1.0, ALU.min,
                                            ALU.add)
                    xc = p_xct.tile([128, SLAB], dt.bfloat16, tag="xcp",
                                    name="xcp", bufs=NCT)
                    nc.vector.scalar_tensor_tensor(xc[:], z[:], 0.0, e[:],
                                                   ALU.max, ALU.add)
                    # layer 2 accumulation: h2 += xcat_ct @ Wo_ct
                    for it in range(NIT):
                        nc.tensor.matmul(ps2l[it][:],
                                         xc[:, it * 128:(it + 1) * 128],
                                         wo_sb[ct][:],
                                         start=(ct == 0), stop=(ct == NCT - 1))

                # layer-2 weights w2 = exp(s2') with NO max subtraction:
                # s2' stays well under fp32/bf16 exp range and the common
                # scale cancels exactly in num/den.
                for it in range(NIT):
                    rows = slice((it % 2) * 128, (it % 2 + 1) * 128)
                    w2 = p_l2a.tile([128, 1], dt.float32, tag="w2", name="w2",
                                    bufs=2)
                    nc.scalar.activation(w2[:], ps2l[it][:, NCLS:G2C], AF.Exp)
                    g2 = p_l2a.tile([128, PAD2], dt.float32, tag="g2", name="g2",
                                    bufs=2)
                    nc.vector.tensor_scalar_mul(g2[:, 0:NCLS],
                                                ps2l[it][:, 0:NCLS], w2[:])
                    nc.vector.tensor_copy(g2[:, NCLS:G2C], w2[:])
                    nc.vector.memset(g2[:, G2C:PAD2], 0.0)
                    g2b = p_l2a.tile([128, PAD2], dt.bfloat16, tag="g2b",
                                     name="g2b", bufs=2)
                    nc.vector.tensor_copy(g2b[:], g2[:])
                    nc.sync.dma_start(g2_slab[it // 2][rows, :], g2b[:])
                    if it == 1:
                        nc.gpsimd.collective_compute(
                            "AllGather", ALU.bypass, replica_groups=rg,
                            ins=[g2_slab[0][:]], outs=[g2_full[0][:]])
                nc.gpsimd.collective_compute(
                    "AllGather", ALU.bypass, replica_groups=rg,
                    ins=[g2_slab[1][:]], outs=[g2_full[1][:]])

            # L2 adjacency matmul + final epilogue
            with (
                tc.tile_pool(name="g2t", bufs=NJT) as p_g2t,
                tc.tile_pool(name="fin", bufs=1) as p_f,
                tc.tile_pool(name="ps2", bufs=4, space="PSUM") as ps_2,
            ):
                g2tiles = [[], []]
                for k in range(2):
                    g2v = g2_full[k].rearrange("(t p) c -> t p c", p=128)
                    for t in range(NJT // 2):
                        gt2 = p_g2t.tile([128, PAD2], dt.bfloat16, tag="g2t",
                                         name="g2t")
                        eng = nc.sync if t % 2 == 0 else nc.scalar
                        eng.dma_start(gt2[:], g2v[t])
                        g2tiles[k].append(gt2)
                ps2 = [ps_2.tile([128, G2C], dt.float32, tag="ps2", name="ps2")
                       for _ in range(NIT)]
                for k in range(2):
                    for it in range(NIT):
                        for t in range(NJT // 2):
                            jt = (t // 2) * 4 + k * 2 + (t % 2)
                            lhs = adjs(jt, it * 128, (it + 1) * 128)
                            nc.tensor.matmul(ps2[it][:],
                                             lhs, g2tiles[k][t][:, 0:G2C],
                                             start=(k == 0 and t == 0),
                                             stop=(k == 1 and t == NJT // 2 - 1))
                for it in range(NIT):
                    r2 = p_f.tile([128, 1], dt.float32, tag="r2", name="r2",
                                  bufs=2)
                    nc.vector.reciprocal(r2[:], ps2[it][:, NCLS:G2C])
                    z = p_f.tile([128, NCLS], dt.float32, tag="z2", name="z2",
                                 bufs=2)
                    nc.vector.tensor_scalar_mul(z[:], ps2[it][:, 0:NCLS], r2[:])
                    e = p_f.tile([128, NCLS], dt.float32, tag="e2", name="e2",
                                 bufs=2)
                    nc.scalar.activation(e[:], z[:], AF.Exp)
                    nc.vector.tensor_scalar(e[:], e[:], 1.0, -1.0, ALU.min,
                                            ALU.add)
                    o = p_f.tile([128, NCLS], dt.float32, tag="o2", name="o2",
                                 bufs=2)
                    nc.vector.scalar_tensor_tensor(o[:], z[:], 0.0, e[:],
                                                   ALU.max, ALU.add)
                    negm = p_f.tile([128, 1], dt.float32, tag="negm", name="negm",
                                    bufs=2)
                    nc.vector.tensor_reduce(negm[:], o[:],
                                            axis=mybir.AxisListType.X,
                                            op=ALU.max, negate=True)
                    t = p_f.tile([128, NCLS], dt.float32, tag="texp", name="texp",
                                 bufs=2)
                    nc.scalar.activation(t[:], o[:], AF.Exp, bias=negm[:])
                    ssum = p_f.tile([128, 1], dt.float32, tag="ssum", name="ssum",
                                    bufs=2)
                    nc.vector.tensor_reduce(ssum[:], t[:],
                                            axis=mybir.AxisListType.X,
                                            op=ALU.add)
                    lg = p_f.tile([128, 1], dt.float32, tag="lg", name="lg",
                                  bufs=2)
                    nc.scalar.activation(lg[:], ssum[:], AF.Ln)
                    fin = p_f.tile([128, NCLS], dt.float32, tag="fin", name="fin",
                                   bufs=2)
                    nc.vector.tensor_scalar(fin[:], o[:], negm[:], lg[:],
                                            ALU.add, ALU.subtract)
                    nc.sync.dma_start(out_d[it * 128:(it + 1) * 128, :], fin[:])

    nc.finalize()
    return nc


_CACHE = {}


def _pair(a):
    hi = a.astype(BF16)
    lo = (a - hi.astype(np.float32)).astype(BF16)
    return hi, lo


def prepare_inputs(x, adj, W_heads, a_heads, W_out, a_out):
    """Shard + lay out the full inputs for the 8 cores."""
    x2 = np.asarray(x, np.float32)[0]          # [N, F]
    adj2 = np.asarray(adj)[0]                  # [N, N] int32
    W3 = np.asarray(W_heads, np.float32).reshape(NH, F, HID)
    a3 = np.asarray(a_heads, np.float32)       # [NH, 2*HID, 1]
    Wo = np.asarray(W_out, np.float32).reshape(GH_TOT, NCLS)
    ao = np.asarray(a_out, np.float32)         # [2*NCLS, 1]

    # fold the edge-score projections into the weights:
    #   s2 = x @ (W @ a2),   s2' = xcat @ (Wo @ ao2)
    u = np.einsum("hfo,ho->hf", W3.astype(np.float64),
                  a3[:, HID:, 0].astype(np.float64)).astype(np.float32)  # [NH,F]
    u_hi, u_lo = _pair(u)
    U6 = np.zeros((F, 8), BF16)
    U3 = np.zeros((F, 8), BF16)
    for h in range(NH):
        U6[:, 2 * h] = u_hi[h]
        U6[:, 2 * h + 1] = u_lo[h]
        U3[:, h] = u_hi[h]
    u2 = (Wo.astype(np.float64) @ ao[NCLS:, 0].astype(np.float64)).astype(np.float32)
    Wo_ext = np.concatenate([Wo, u2[:, None]], axis=1)       # [GH, 257]
    Wo_b = Wo_ext.astype(BF16)
    W_b = W3.astype(BF16)
    xT = np.ascontiguousarray(x2.T)            # [F, N]
    adjb = adj2.astype(BF16)                   # exact 0/1

    # exact per-head max of s2 = x @ u, folded on the host so the device
    # needs no max-reduction collective.  Mirror the device arithmetic
    # (bf16 hi/lo pair dot products accumulated in fp32).
    xh_f, xl_f = _pair(x2)
    s2 = (xh_f.astype(np.float32) @ u_hi.T.astype(np.float32)
          + xh_f.astype(np.float32) @ u_lo.T.astype(np.float32)
          + xl_f.astype(np.float32) @ u_hi.T.astype(np.float32))  # [N, NH]
    negC = -s2.max(axis=0, keepdims=True).astype(np.float32)      # [1, NH]

    in_maps = []
    for c in range(NCORES):
        sl = slice(c * SLAB, (c + 1) * SLAB)
        xh, xl = _pair(np.ascontiguousarray(xT[:, sl]))
        in_maps.append({
            "adjT": np.ascontiguousarray(adjb[sl, :].T),
            "xT_hi": xh, "xT_lo": xl,
            "U6": U6, "U3": U3, "negC": negC,
            "W": W_b, "Wo": Wo_b,
        })
    return in_maps


def kernel(x, adj, W_heads, a_heads, W_out, a_out):
    if "nc" not in _CACHE:
        # touch the devices once so any residual bad state from a previous
        # process surfaces (and clears) before the real run
        try:
            import jax
            jax.block_until_ready(jax.numpy.zeros(8))
        except Exception:
            pass
        _CACHE["nc"] = build()
    nc = _CACHE["nc"]
    in_maps = prepare_inputs(x, adj, W_heads, a_heads, W_out, a_out)
    res = run_bass_kernel_spmd(nc, in_maps, list(range(NCORES)))
    out = np.concatenate([res.results[c]["out"] for c in range(NCORES)], axis=0)
    return out.reshape(1, N, NCLS)


# revision 36
# speedup vs baseline: 2.0663x; 1.0249x over previous
"""GAT (2-layer, 3-head) forward on 8 Trainium2 NeuronCores.

Math: with LeakyReLU slope ALPHA=1.0 the edge score e_ij = s1_i + s2_j is
linear, and s1_i cancels inside the row softmax.  The masked softmax over
j therefore reduces to column weights w_j = exp(s2_j - C) restricted to
adj, giving

    h'_i = (sum_j adj_ij * w_j * h_j) / (sum_j adj_ij * w_j)

i.e. one adjacency matmul against G = [w*h | w].  Both GAT layers take
this form (the same adjacency masks both), so the whole network is two
A-matmuls plus small projections.

Sharding: rows of h' (nodes) across 8 cores; each core holds lhsT-layout
adjacency columns A^T[:, slab] and computes its 512-row slab.  Matmuls
run in single bf16 (the tolerance is 2e-2; only the edge-score s2, which
sits in an exponent, is kept in ~fp32 via a hi/lo pair trick folded into
the input prep).  G is gathered in two chunks: [G0 | w] first so the
denominator and head-0 tiles unblock early, then [G1 | G2].
"""
import sys

sys.path.insert(0, "/opt/trn_rl_repo")

import numpy as np
import ml_dtypes

import concourse.bass as bass
import concourse.bacc as bacc
import concourse.mybir as mybir
import concourse.bass_isa as bass_isa
import concourse.tile as tile
from concourse.bass_utils import run_bass_kernel_spmd

BF16 = ml_dtypes.bfloat16

N = 4096
F = 768
HID = 768
NH = 3
NCLS = 256
NCORES = 8
SLAB = N // NCORES          # 512 rows per core
NIT = SLAB // 128           # 4 i-tiles per core
NJT = N // 128              # 32 j-tiles
NFT = F // 128              # 6 f-tiles
NCT = NH * NFT              # 18 feature col-tiles of G
G2C = NCLS + 1              # 257 = classes + s2' column (folded u2)
PAD2 = 264                  # G2 padded to 32B rows
WCOLS = 16                  # w-column slab width (3 used + pad)
GA = WCOLS + HID            # gather-A width: [w cols | head0 G]
GB = 2 * HID                # gather-B width: head1 + head2 G
GH_TOT = NH * HID           # 2304 xcat feature rows of Wo

AF = mybir.ActivationFunctionType
ALU = mybir.AluOpType


def _enable_ldw_opt():
    # walrus defaults to --enable-ldw-opt=false; with it off every LDWEIGHTS
    # serializes against the previous matmul (~427ns vs ~213ns per 512-col
    # matmul).  Patch the arg builder so the stationary loads pipeline.
    import concourse.bass_utils as _bu
    if getattr(_bu, "_ldw_opt_patched", False):
        return
    _orig = _bu.get_walrus_args

    def _patched(*a, **k):
        args = _orig(*a, **k)
        return [x.replace("--enable-ldw-opt=false", "--enable-ldw-opt=true")
                for x in args]

    _bu.get_walrus_args = _patched
    _bu._ldw_opt_patched = True


def build():
    dt = mybir.dt
    _enable_ldw_opt()
    nc = bacc.Bacc(num_devices=NCORES)

    adjT_d = nc.dram_tensor("adjT", [N, SLAB], dt.bfloat16, kind="ExternalInput")
    xTh_d = nc.dram_tensor("xT_hi", [F, SLAB], dt.bfloat16, kind="ExternalInput")
    U6_d = nc.dram_tensor("U6", [F, 8], dt.bfloat16, kind="ExternalInput")
    # negC[0, h] = -max_i s2_i(head h), computed exactly on the host
    negC_d = nc.dram_tensor("negC", [1, NH], dt.float32, kind="ExternalInput")
    W_d = nc.dram_tensor("W", [NH, F, HID], dt.bfloat16, kind="ExternalInput")
    Wo_d = nc.dram_tensor("Wo", [GH_TOT, G2C], dt.bfloat16, kind="ExternalInput")
    out_d = nc.dram_tensor("out", [SLAB, NCLS], dt.float32, kind="ExternalOutput")

    # DRAM scratch + collective buffers
    gsA = nc.dram_tensor("gsA", [SLAB, GA], dt.bfloat16)
    gfA = nc.dram_tensor("gfA", [N, GA], dt.bfloat16, addr_space="Shared")
    gsB = nc.dram_tensor("gsB", [SLAB, GB], dt.bfloat16)
    gfB = nc.dram_tensor("gfB", [N, GB], dt.bfloat16, addr_space="Shared")
    # g2 gathered in two half-slab chunks so the L2 matmul can start on the
    # first half while the second is in flight
    g2_slab = [nc.dram_tensor(f"g2_slab{k}", [SLAB // 2, PAD2], dt.bfloat16)
               for k in range(2)]
    g2_full = [nc.dram_tensor(f"g2_full{k}", [N // 2, PAD2], dt.bfloat16,
                              addr_space="Shared") for k in range(2)]

    rg = [list(range(NCORES))]

    with tile.TileContext(nc) as tc:
      with tc.tile_pool(name="adjt", bufs=NJT) as p_adjt:
        # ---------------- phase 1: s2, w, h=x@W, G build + gathers ----------
        with (
            tc.tile_pool(name="xw", bufs=1) as p_xw,
            tc.tile_pool(name="small", bufs=1) as p_sm,
            tc.tile_pool(name="gtmp", bufs=1) as p_gt,
        ):
            # Batched input loads: one big DMA per tensor (chunked transfers
            # serialize at ~650ns per 128KB, so 70 small DMAs would cost
            # ~45us of serial load time).  x + head-0 W first: they gate
            # s2 and the first x@W matmuls.
            xh_all = p_xw.tile([128, NFT, SLAB], dt.bfloat16, tag="xh", name="xh")
            nc.sync.dma_start(xh_all[:],
                              xTh_d.rearrange("(ft p) i -> p ft i", p=128))

            def xhi(ft, c0, c1):
                return xh_all[:, ft, c0:c1]

            u6 = p_sm.tile([128, NFT, 8], dt.bfloat16, tag="u6", name="u6")
            nc.gpsimd.dma_start(u6[:], U6_d.rearrange("(ft p) c -> p ft c", p=128))
            negC = p_sm.tile([1, NH], dt.float32, tag="negC", name="negC")
            nc.gpsimd.dma_start(negC[:], negC_d[:])
            negCbc = p_sm.tile([128, NH], dt.float32, tag="negCbc", name="negCbc")
            nc.gpsimd.partition_broadcast(negCbc[:], negC[:], channels=128)

            W_t = W_d.rearrange("h (ft p) o -> p h ft o", p=128)
            w0_all = p_xw.tile([128, NFT, HID], dt.bfloat16, tag="w0", name="w0")
            nc.sync.dma_start(w0_all[:], W_t[:, 0])
            w12_all = p_xw.tile([128, 2, NFT, HID], dt.bfloat16, tag="w12",
                                name="w12")
            nc.scalar.dma_start(w12_all[:], W_t[:, 1:3])

            def wsl(h, ft, c0, c1):
                if h == 0:
                    return w0_all[:, ft, c0:c1]
                return w12_all[:, h - 1, ft, c0:c1]

            adjt_all = []
            adjT_t = adjT_d.rearrange("(half jh p) i -> half p jh i",
                                      half=2, p=128)
            for half in range(2):
                t = p_adjt.tile([128, NJT // 2, SLAB], dt.bfloat16, tag="adjt",
                                name="adjt", bufs=2)
                eng = nc.sync if half == 0 else nc.scalar
                eng.dma_start(t[:], adjT_t[half])
                adjt_all.append(t)

            def adjs(j, c0=0, c1=SLAB):
                return adjt_all[j // (NJT // 2)][:, j % (NJT // 2), c0:c1]

            # s2 = x_hi @ (u_hi + u_lo): one PSUM bank, no inter-it reuse
            # stalls.  u kept as a bf16 pair; x_hi-only costs ~0.8% on w,
            # which averages out over ~2k neighbours.
            s2_sb = []
            for h in range(NH):
                s2_sb.append(p_sm.tile([128, NIT], dt.float32, tag="s2",
                                       name="s2", bufs=NH))
            with tc.tile_pool(name="psS", bufs=1, space="PSUM") as ps_s:
                p6 = ps_s.tile([128, NIT, 8], dt.float32, tag="p6", name="p6")
                for it in range(NIT):
                    for ft in range(NFT):
                        xh = xhi(ft, it * 128, (it + 1) * 128)
                        nc.tensor.matmul(p6[:, it, :], xh, u6[:, ft, :],
                                         start=(ft == 0), stop=(ft == NFT - 1))
                for it in range(NIT):
                    t6 = p_sm.tile([128, 8], dt.float32, tag="t6", name="t6",
                                   bufs=2)
                    nc.vector.tensor_copy(t6[:], p6[:, it, :])
                    tsum = p_sm.tile([128, NH], dt.float32, tag="tsum",
                                     name="tsum", bufs=2)
                    nc.vector.tensor_tensor(tsum[:], t6[:, 0:2 * NH:2],
                                            t6[:, 1:2 * NH:2], ALU.add)
                    for h in range(NH):
                        nc.vector.tensor_copy(s2_sb[h][:, it:it + 1],
                                              tsum[:, h:h + 1])

            # w = exp(s2 - C) with the host-computed C — no collective needed
            w_sb = []
            for h in range(NH):
                w = p_sm.tile([128, NIT], dt.float32, tag="wexp", name="wexp",
                              bufs=NH)
                nc.scalar.activation(w[:], s2_sb[h][:], AF.Exp,
                                     bias=negCbc[:, h:h + 1])
                w_sb.append(w)
            wb3 = p_sm.tile([128, NH, NIT], dt.bfloat16, tag="wb3", name="wb3")
            for h in range(NH):
                nc.vector.tensor_copy(wb3[:, h, :], w_sb[h][:])
            for it in range(NIT):
                rows = slice(it * 128, (it + 1) * 128)
                wt = p_sm.tile([128, WCOLS], dt.bfloat16, tag="wt", name="wt",
                               bufs=2)
                nc.vector.memset(wt[:], 0.0)
                nc.vector.tensor_copy(wt[:, 0:NH], wb3[:, :, it])
                nc.gpsimd.dma_start(gsA[rows, 0:WCOLS], wt[:])

            # h = x@W, scaled to G as each tile drains; gather A as soon as
            # head 0 is done, so the denominator and head-0 tiles unblock.
            with tc.tile_pool(name="psA", bufs=2, space="PSUM") as ps_a:
                for h in range(NH):
                    for it in range(NIT):
                        ps = ps_a.tile([128, HID], dt.float32, tag="psA", name="psA")
                        for ft in range(NFT):
                            xh = xhi(ft, it * 128, (it + 1) * 128)
                            nc.tensor.matmul(ps[:, 0:512], xh, wsl(h, ft, 0, 512),
                                             start=(ft == 0), stop=(ft == NFT - 1))
                            nc.tensor.matmul(ps[:, 512:HID], xh,
                                             wsl(h, ft, 512, HID),
                                             start=(ft == 0), stop=(ft == NFT - 1))
                        g = p_gt.tile([128, HID], dt.bfloat16, tag="g0",
                                      name="g0", bufs=4)
                        nc.vector.tensor_scalar_mul(g[:], ps[:],
                                                    w_sb[h][:, it:it + 1])
                        rows = slice(it * 128, (it + 1) * 128)
                        if h == 0:
                            nc.sync.dma_start(gsA[rows, WCOLS:GA], g[:])
                        else:
                            eng = nc.sync if h == 1 else nc.scalar
                            eng.dma_start(gsB[rows, (h - 1) * HID:h * HID], g[:])
                    if h == 0:
                        nc.gpsimd.collective_compute(
                            "AllGather", ALU.bypass, replica_groups=rg,
                            ins=[gsA[:]], outs=[gfA[:]])
                nc.gpsimd.collective_compute(
                    "AllGather", ALU.bypass, replica_groups=rg,
                    ins=[gsB[:]], outs=[gfB[:]])

        # ---------------- L1 adjacency matmul + epilogue + layer 2 ----------
        with tc.tile_pool(name="xct", bufs=1) as p_xct:
            with (
                tc.tile_pool(name="gst", bufs=8) as p_gst,
                tc.tile_pool(name="etmp", bufs=1) as p_et,
                tc.tile_pool(name="wo", bufs=1) as p_wo,
                tc.tile_pool(name="l2a", bufs=1) as p_l2a,
                tc.tile_pool(name="ps1", bufs=4, space="PSUM") as ps_1,
                tc.tile_pool(name="psh2", bufs=4, space="PSUM") as ps_h2,
            ):
                # Wo loads early; they only feed the inline h2 matmuls
                wo_sb = []
                Wo_t = Wo_d.rearrange("(ot p) c -> ot p c", p=128)
                for ot in range(NCT):
                    t = p_wo.tile([128, G2C], dt.bfloat16, tag="wo", name="wo",
                                  bufs=NCT)
                    eng = nc.sync if ot % 2 == 0 else nc.scalar
                    eng.dma_start(t[:], Wo_t[ot])
                    wo_sb.append(t)

                # feature col-tiles, head-major; epilogue + h2 inline per ct.
                # ct 0 also carries the w columns (first WCOLS of gfA), so the
                # denominator matmuls ride its tile loads — no separate
                # strided gather of w.
                gvA = gfA.rearrange("(jb q p) c -> jb p q c", q=4, p=128)
                gvB = gfB.rearrange("(jb q p) c -> jb p q c", q=4, p=128)
                ps2l = [ps_h2.tile([128, G2C], dt.float32, tag="psh2",
                                   name="psh2") for _ in range(NIT)]
                rbc = [None] * NH
                psd = ps_1.tile([128, SLAB], dt.float32, tag="psd", name="psd",
                                bufs=1)
                for ct in range(NCT):
                    h = ct // NFT
                    lct = ct % NFT
                    ps = ps_1.tile([128, SLAB], dt.float32, tag="ps1", name="ps1",
                                   bufs=3)
                    for jb in range(NJT // 4):
                        if ct == 0:
                            gt = p_gst.tile([128, 4, WCOLS + 128], dt.bfloat16,
                                            tag="gst0", name="gst0", bufs=8)
                            eng = nc.sync if jb % 2 == 0 else nc.scalar
                            eng.dma_start(gt[:], gvA[jb, :, :, 0:WCOLS + 128])
                            goff = WCOLS
                        elif h == 0:
                            gt = p_gst.tile([128, 4, 128], dt.bfloat16,
                                            tag="gst", name="gst")
                            eng = nc.sync if jb % 2 == 0 else nc.scalar
                            eng.dma_start(gt[:], gvA[jb, :, :,
                                                     WCOLS + lct * 128:
                                                     WCOLS + (lct + 1) * 128])
                            goff = 0
                        else:
                            gt = p_gst.tile([128, 4, 128], dt.bfloat16,
                                            tag="gst", name="gst")
                            eng = nc.sync if jb % 2 == 0 else nc.scalar
                            c0 = (h - 1) * HID + lct * 128
                            eng.dma_start(gt[:], gvB[jb, :, :, c0:c0 + 128])
                            goff = 0
                        for q in range(4):
                            j = jb * 4 + q
                            if ct == 0:
                                nc.tensor.matmul(psd[0:NH, :], gt[:, q, 0:NH],
                                                 adjs(j), start=(j == 0),
                                                 stop=(j == NJT - 1))
                            nc.tensor.matmul(ps[:], gt[:, q, goff:goff + 128],
                                             adjs(j), start=(j == 0),
                                             stop=(j == NJT - 1))
                    if ct == 0:
                        recip3 = p_et.tile([NH, SLAB], dt.float32, tag="recip3",
                                           name="recip3")
                        nc.vector.reciprocal(recip3[:], psd[0:NH, :])
                        for hh in range(NH):
                            rrow = p_et.tile([1, SLAB], dt.float32, tag="rrow",
                                             name="rrow", bufs=2)
                            nc.sync.dma_start(rrow[:], recip3[hh:hh + 1, :])
                            rb = p_et.tile([128, SLAB], dt.float32, tag="rbc",
                                           name="rbc", bufs=NH)
                            nc.gpsimd.partition_broadcast(rb[:], rrow[:],
                                                          channels=128)
                            rbc[hh] = rb
                    # xcatT tile = elu(numT / den), bf16
                    z = p_et.tile([128, SLAB], dt.float32, tag="z", name="z",
                                  bufs=2)
                    nc.vector.tensor_tensor(z[:], ps[:], rbc[h][:], ALU.mult)
                    e = p_et.tile([128, SLAB], dt.float32, tag="e", name="e",
                                  bufs=2)
                    nc.scalar.activation(e[:], z[:], AF.Exp)
                    nc.vector.tensor_scalar(e[:], e[:], 1.0, -1.0, ALU.min,
                                            ALU.add)
                    xc = p_xct.tile([128, SLAB], dt.bfloat16, tag="xcp",
                                    name="xcp", bufs=NCT)
                    nc.vector.scalar_tensor_tensor(xc[:], z[:], 0.0, e[:],
                                                   ALU.max, ALU.add)
                    # layer 2 accumulation: h2 += xcat_ct @ Wo_ct
                    for it in range(NIT):
                        nc.tensor.matmul(ps2l[it][:],
                                         xc[:, it * 128:(it + 1) * 128],
                                         wo_sb[ct][:],
                                         start=(ct == 0), stop=(ct == NCT - 1))

                # layer-2 weights w2 = exp(s2') with NO max subtraction:
                # s2' stays well under fp32/bf16 exp range and the common
                # scale cancels exactly in num/den.
                for it in range(NIT):
                    rows = slice((it % 2) * 128, (it % 2 + 1) * 128)
                    w2 = p_l2a.tile([128, 1], dt.float32, tag="w2", name="w2",
                                    bufs=2)
                    nc.scalar.activation(w2[:], ps2l[it][:, NCLS:G2C], AF.Exp)
                    g2b = p_l2a.tile([128, PAD2], dt.bfloat16, tag="g2b",
                                     name="g2b", bufs=2)
                    nc.vector.tensor_scalar_mul(g2b[:, 0:NCLS],
                                                ps2l[it][:, 0:NCLS], w2[:])
                    nc.vector.tensor_copy(g2b[:, NCLS:G2C], w2[:])
                    nc.vector.memset(g2b[:, G2C:PAD2], 0.0)
                    nc.sync.dma_start(g2_slab[it // 2][rows, :], g2b[:])
                    if it == 1:
                        nc.gpsimd.collective_compute(
                            "AllGather", ALU.bypass, replica_groups=rg,
                            ins=[g2_slab[0][:]], outs=[g2_full[0][:]])
                nc.gpsimd.collective_compute(
                    "AllGather", ALU.bypass, replica_groups=rg,
                    ins=[g2_slab[1][:]], outs=[g2_full[1][:]])

            # L2 adjacency matmul + final epilogue
            with (
                tc.tile_pool(name="g2t", bufs=NJT) as p_g2t,
                tc.tile_pool(name="fin", bufs=1) as p_f,
                tc.tile_pool(name="ps2", bufs=4, space="PSUM") as ps_2,
            ):
                g2tiles = [[], []]
                for k in range(2):
                    g2v = g2_full[k].rearrange("(t p) c -> t p c", p=128)
                    for t in range(NJT // 2):
                        gt2 = p_g2t.tile([128, PAD2], dt.bfloat16, tag="g2t",
                                         name="g2t")
                        eng = nc.sync if t % 2 == 0 else nc.scalar
                        eng.dma_start(gt2[:], g2v[t])
                        g2tiles[k].append(gt2)
                ps2 = [ps_2.tile([128, G2C], dt.float32, tag="ps2", name="ps2")
                       for _ in range(NIT)]
                for k in range(2):
                    for it in range(NIT):
                        for t in range(NJT // 2):
                            jt = (t // 2) * 4 + k * 2 + (t % 2)
                            lhs = adjs(jt, it * 128, (it + 1) * 128)
                            nc.tensor.matmul(ps2[it][:],
                                             lhs, g2tiles[k][t][:, 0:G2C],
                                             start=(k == 0 and t == 0),
                                             stop=(k == 1 and t == NJT // 2 - 1))
                for it in range(NIT):
                    r2 = p_f.tile([128, 1], dt.float32, tag="r2", name="r2",
                                  bufs=2)
                    nc.vector.reciprocal(r2[:], ps2[it][:, NCLS:G2C])
                    z = p_f.tile([128, NCLS], dt.float32, tag="z2", name="z2",
                                 bufs=2)
                    nc.vector.tensor_scalar_mul(z[:], ps2[it][:, 0:NCLS], r2[:])
                    e = p_f.tile([128, NCLS], dt.float32, tag="e2", name="e2",
                                 bufs=2)
                    nc.scalar.activation(e[:], z[:], AF.Exp)
                    nc.vector.tensor_scalar(e[:], e[:], 1.0, -1.0, ALU.min,
                                            ALU.add)
                    o = p_f.tile([128, NCLS], dt.float32, tag="o2", name="o2",
                                 bufs=2)
                    nc.vector.scalar_tensor_tensor(o[:], z[:], 0.0, e[:],
                                                   ALU.max, ALU.add)
                    # log_softmax without max subtraction: o <= ~10, so
                    # exp stays comfortably inside fp32 range
                    t = p_f.tile([128, NCLS], dt.float32, tag="texp", name="texp",
                                 bufs=2)
                    nc.scalar.activation(t[:], o[:], AF.Exp)
                    ssum = p_f.tile([128, 1], dt.float32, tag="ssum", name="ssum",
                                    bufs=2)
                    nc.vector.tensor_reduce(ssum[:], t[:],
                                            axis=mybir.AxisListType.X,
                                            op=ALU.add)
                    lg = p_f.tile([128, 1], dt.float32, tag="lg", name="lg",
                                  bufs=2)
                    nc.scalar.activation(lg[:], ssum[:], AF.Ln)
                    fin = p_f.tile([128, NCLS], dt.float32, tag="fin", name="fin",
                                   bufs=2)
                    nc.vector.tensor_scalar(fin[:], o[:], lg[:], None,
                                            ALU.subtract)
                    nc.sync.dma_start(out_d[it * 128:(it + 1) * 128, :], fin[:])

    nc.finalize()
    return nc


_CACHE = {}


def _pair(a):
    hi = a.astype(BF16)
    lo = (a - hi.astype(np.float32)).astype(BF16)
    return hi, lo


def prepare_inputs(x, adj, W_heads, a_heads, W_out, a_out):
    """Shard + lay out the full inputs for the 8 cores."""
    x2 = np.asarray(x, np.float32)[0]          # [N, F]
    adj2 = np.asarray(adj)[0]                  # [N, N] int32
    W3 = np.asarray(W_heads, np.float32).reshape(NH, F, HID)
    a3 = np.asarray(a_heads, np.float32)       # [NH, 2*HID, 1]
    Wo = np.asarray(W_out, np.float32).reshape(GH_TOT, NCLS)
    ao = np.asarray(a_out, np.float32)         # [2*NCLS, 1]

    # fold the edge-score projections into the weights:
    #   s2 = x @ (W @ a2),   s2' = xcat @ (Wo @ ao2)
    u = np.einsum("hfo,ho->hf", W3.astype(np.float64),
                  a3[:, HID:, 0].astype(np.float64)).astype(np.float32)  # [NH,F]
    u_hi, u_lo = _pair(u)
    U6 = np.zeros((F, 8), BF16)
    for h in range(NH):
        U6[:, 2 * h] = u_hi[h]
        U6[:, 2 * h + 1] = u_lo[h]
    u2 = (Wo.astype(np.float64) @ ao[NCLS:, 0].astype(np.float64)).astype(np.float32)
    Wo_ext = np.concatenate([Wo, u2[:, None]], axis=1)       # [GH, 257]
    Wo_b = Wo_ext.astype(BF16)
    W_b = W3.astype(BF16)
    xT = np.ascontiguousarray(x2.T)            # [F, N]
    adjb = adj2.astype(BF16)                   # exact 0/1

    # exact per-head max of s2 = x @ u, folded on the host so the device
    # needs no max-reduction collective.  Mirror the device arithmetic
    # (bf16 x_hi against the u hi/lo pair, accumulated in fp32).
    xh_f = x2.astype(BF16).astype(np.float32)
    s2 = (xh_f @ u_hi.T.astype(np.float32)
          + xh_f @ u_lo.T.astype(np.float32))                     # [N, NH]
    negC = -s2.max(axis=0, keepdims=True).astype(np.float32)      # [1, NH]

    in_maps = []
    for c in range(NCORES):
        sl = slice(c * SLAB, (c + 1) * SLAB)
        xh = np.ascontiguousarray(xT[:, sl]).astype(BF16)
        in_maps.append({
            "adjT": np.ascontiguousarray(adjb[sl, :].T),
            "xT_hi": xh,
            "U6": U6, "negC": negC,
            "W": W_b, "Wo": Wo_b,
        })
    return in_maps


def kernel(x, adj, W_heads, a_heads, W_out, a_out):
    if "nc" not in _CACHE:
        # touch the devices once so any residual bad state from a previous
        # process surfaces (and clears) before the real run
        try:
            import jax
            jax.block_until_ready(jax.numpy.zeros(8))
        except Exception:
            pass
        _CACHE["nc"] = build()
    nc = _CACHE["nc"]
    in_maps = prepare_inputs(x, adj, W_heads, a_heads, W_out, a_out)
    res = run_bass_kernel_spmd(nc, in_maps, list(range(NCORES)))
    out = np.concatenate([res.results[c]["out"] for c in range(NCORES)], axis=0)
    return out.reshape(1, N, NCLS)


# revision 49
# speedup vs baseline: 2.4392x; 1.1805x over previous
"""GAT (2-layer, 3-head) forward on 8 Trainium2 NeuronCores.

Math: with LeakyReLU slope ALPHA=1.0 the edge score e_ij = s1_i + s2_j is
linear, and s1_i cancels inside the row softmax.  The masked softmax over
j therefore reduces to column weights w_j = exp(s2_j - C) restricted to
adj, giving

    h'_i = (sum_j adj_ij * w_j * h_j) / (sum_j adj_ij * w_j)

i.e. one adjacency matmul against G = [w*h | w].  Both GAT layers take
this form (the same adjacency masks both), so the whole network is two
A-matmuls plus small projections.

Sharding: rows of h' (nodes) across 8 cores; each core holds lhsT-layout
adjacency columns A^T[:, slab] and computes its 512-row slab.  Matmuls
run in single bf16 (the tolerance is 2e-2; only the edge-score s2, which
sits in an exponent, is kept in ~fp32 via a hi/lo pair trick folded into
the input prep).  G is gathered in two chunks: [G0 | w] first so the
denominator and head-0 tiles unblock early, then [G1 | G2].
"""
import sys

sys.path.insert(0, "/opt/trn_rl_repo")

import numpy as np
import ml_dtypes

import concourse.bass as bass
import concourse.bacc as bacc
import concourse.mybir as mybir
import concourse.bass_isa as bass_isa
import concourse.tile as tile
from concourse.bass_utils import run_bass_kernel_spmd

BF16 = ml_dtypes.bfloat16
F8E4 = ml_dtypes.float8_e4m3

N = 4096
F = 768
HID = 768
NH = 3
NCLS = 256
NCORES = 8
SLAB = N // NCORES          # 512 rows per core
NIT = SLAB // 128           # 4 i-tiles per core
NJT = N // 128              # 32 j-tiles
NFT = F // 128              # 6 f-tiles
NCT = NH * NFT              # 18 feature col-tiles of G
G2C = NCLS + 1              # 257 = classes + s2' column (folded u2)
PAD2 = 264                  # G2 padded to 32B rows
WCOLS = 32                  # w-column slab width (6 used + pad, 32B rows)
GA = WCOLS + HID            # gather-A width: [w cols | head0 G]
GB = 2 * HID                # gather-B width: head1 + head2 G
GH_TOT = NH * HID           # 2304 xcat feature rows of Wo
SG = 8.0                    # fp8 scale on G ( |G*8| << 240 )
SW = 128.0                  # fp8 scale on w (w <= 1)
NJJ = NJT // 2              # 16 j-pair blocks for DoubleRow

AF = mybir.ActivationFunctionType
ALU = mybir.AluOpType


def _enable_ldw_opt():
    # walrus defaults to --enable-ldw-opt=false; with it off every LDWEIGHTS
    # serializes against the previous matmul (~427ns vs ~213ns per 512-col
    # matmul).  Patch the arg builder so the stationary loads pipeline.
    import concourse.bass_utils as _bu
    if getattr(_bu, "_ldw_opt_patched", False):
        return
    _orig = _bu.get_walrus_args

    def _patched(*a, **k):
        args = _orig(*a, **k)
        return [x.replace("--enable-ldw-opt=false", "--enable-ldw-opt=true")
                for x in args]

    _bu.get_walrus_args = _patched
    _bu._ldw_opt_patched = True


def build():
    dt = mybir.dt
    _enable_ldw_opt()
    nc = bacc.Bacc(num_devices=NCORES)

    adjT8_d = nc.dram_tensor("adjT8", [N, SLAB], dt.float8e4, kind="ExternalInput")
    adjT_d = nc.dram_tensor("adjT", [N, SLAB], dt.bfloat16, kind="ExternalInput")
    xTh_d = nc.dram_tensor("xT_hi", [F, SLAB], dt.bfloat16, kind="ExternalInput")
    U6_d = nc.dram_tensor("U6", [F, 8], dt.bfloat16, kind="ExternalInput")
    # negC[0, h] = -max_i s2_i(head h), computed exactly on the host
    negC_d = nc.dram_tensor("negC", [1, NH], dt.float32, kind="ExternalInput")
    W_d = nc.dram_tensor("W", [NH, F, HID], dt.bfloat16, kind="ExternalInput")
    Wo_d = nc.dram_tensor("Wo", [GH_TOT, G2C], dt.bfloat16, kind="ExternalInput")
    out_d = nc.dram_tensor("out", [SLAB, NCLS], dt.float32, kind="ExternalOutput")

    # DRAM scratch + collective buffers (fp8: halves gather + reload bytes)
    gsA = nc.dram_tensor("gsA", [SLAB, GA], dt.float8e4)
    gfA = nc.dram_tensor("gfA", [N, GA], dt.float8e4, addr_space="Shared")
    gsB = nc.dram_tensor("gsB", [SLAB, GB], dt.float8e4)
    gfB = nc.dram_tensor("gfB", [N, GB], dt.float8e4, addr_space="Shared")
    # g2 gathered in two half-slab chunks so the L2 matmul can start on the
    # first half while the second is in flight
    g2_slab = [nc.dram_tensor(f"g2_slab{k}", [SLAB // 2, PAD2], dt.bfloat16)
               for k in range(2)]
    g2_full = [nc.dram_tensor(f"g2_full{k}", [N // 2, PAD2], dt.bfloat16,
                              addr_space="Shared") for k in range(2)]

    rg = [list(range(NCORES))]

    with tile.TileContext(nc) as tc:
      with tc.tile_pool(name="adjt", bufs=NJT) as p_adjt:
        # ---------------- phase 1: s2, w, h=x@W, G build + gathers ----------
        with (
            tc.tile_pool(name="xw", bufs=1) as p_xw,
            tc.tile_pool(name="small", bufs=1) as p_sm,
            tc.tile_pool(name="gtmp", bufs=1) as p_gt,
        ):
            # Batched input loads: one big DMA per tensor (chunked transfers
            # serialize at ~650ns per 128KB, so 70 small DMAs would cost
            # ~45us of serial load time).  x + head-0 W first: they gate
            # s2 and the first x@W matmuls.
            xh_all = p_xw.tile([128, NFT, SLAB], dt.bfloat16, tag="xh", name="xh")
            nc.sync.dma_start(xh_all[:],
                              xTh_d.rearrange("(ft p) i -> p ft i", p=128))

            def xhi(ft, c0, c1):
                return xh_all[:, ft, c0:c1]

            u6 = p_sm.tile([128, NFT, 8], dt.bfloat16, tag="u6", name="u6")
            nc.gpsimd.dma_start(u6[:], U6_d.rearrange("(ft p) c -> p ft c", p=128))
            negC = p_sm.tile([1, NH], dt.float32, tag="negC", name="negC")
            nc.gpsimd.dma_start(negC[:], negC_d[:])
            negCbc = p_sm.tile([128, NH], dt.float32, tag="negCbc", name="negCbc")
            nc.gpsimd.partition_broadcast(negCbc[:], negC[:], channels=128)

            W_t = W_d.rearrange("h (ft p) o -> p h ft o", p=128)
            w0_all = p_xw.tile([128, NFT, HID], dt.bfloat16, tag="w0", name="w0")
            nc.sync.dma_start(w0_all[:], W_t[:, 0])
            w12_all = p_xw.tile([128, 2, NFT, HID], dt.bfloat16, tag="w12",
                                name="w12")
            nc.scalar.dma_start(w12_all[:], W_t[:, 1:3])

            def wsl(h, ft, c0, c1):
                if h == 0:
                    return w0_all[:, ft, c0:c1]
                return w12_all[:, h - 1, ft, c0:c1]

            # fp8 adjacency, j-pair interleaved for DoubleRow (L1 rhs)
            adj8_all = []
            adjT8_t = adjT8_d.rearrange("(half jj i p) n -> half p jj i n",
                                        half=2, i=2, p=128)
            for half in range(2):
                t = p_adjt.tile([128, NJJ // 2, 2, SLAB], dt.float8e4,
                                tag="adj8", name="adj8", bufs=2)
                eng = nc.sync if half == 0 else nc.scalar
                eng.dma_start(t[:], adjT8_t[half])
                adj8_all.append(t)

            def adjd(jj):
                return adj8_all[jj // (NJJ // 2)][:, jj % (NJJ // 2), :, :]

            # bf16 adjacency per original j-tile (L2 lhsT) — needed only at
            # the tail, loaded after the phase-1 traffic
            adjt_all = []
            adjT_t = adjT_d.rearrange("(half jh p) i -> half p jh i",
                                      half=2, p=128)

            def adjs(j, c0=0, c1=SLAB):
                return adjt_all[j // (NJT // 2)][:, j % (NJT // 2), c0:c1]

            # s2 = x_hi @ (u_hi + u_lo): one PSUM bank, no inter-it reuse
            # stalls.  u kept as a bf16 pair; x_hi-only costs ~0.8% on w,
            # which averages out over ~2k neighbours.
            s2_sb = []
            for h in range(NH):
                s2_sb.append(p_sm.tile([128, NIT], dt.float32, tag="s2",
                                       name="s2", bufs=NH))
            with tc.tile_pool(name="psS", bufs=1, space="PSUM") as ps_s:
                p6 = ps_s.tile([128, NIT, 8], dt.float32, tag="p6", name="p6")
                for it in range(NIT):
                    for ft in range(NFT):
                        xh = xhi(ft, it * 128, (it + 1) * 128)
                        nc.tensor.matmul(p6[:, it, :], xh, u6[:, ft, :],
                                         start=(ft == 0), stop=(ft == NFT - 1))
                for it in range(NIT):
                    t6 = p_sm.tile([128, 8], dt.float32, tag="t6", name="t6",
                                   bufs=2)
                    nc.vector.tensor_copy(t6[:], p6[:, it, :])
                    tsum = p_sm.tile([128, NH], dt.float32, tag="tsum",
                                     name="tsum", bufs=2)
                    nc.vector.tensor_tensor(tsum[:], t6[:, 0:2 * NH:2],
                                            t6[:, 1:2 * NH:2], ALU.add)
                    for h in range(NH):
                        nc.vector.tensor_copy(s2_sb[h][:, it:it + 1],
                                              tsum[:, h:h + 1])

            # w = exp(s2 - C) with the host-computed C — no collective needed.
            # Stage w*SW as an fp8 hi/lo pair (hi + lo/16 ≈ 8 mantissa bits)
            # for the DoubleRow denominator matmul, and keep w*SG in fp32 for
            # scaling G.
            w_sb, w8_sb = [], []
            for h in range(NH):
                w = p_sm.tile([128, NIT], dt.float32, tag="wexp", name="wexp",
                              bufs=NH)
                nc.scalar.activation(w[:], s2_sb[h][:], AF.Exp,
                                     bias=negCbc[:, h:h + 1])
                w_sb.append(w)
                w8 = p_sm.tile([128, NIT], dt.float32, tag="wsg", name="wsg",
                               bufs=NH)
                nc.vector.tensor_scalar_mul(w8[:], w[:], SG)
                w8_sb.append(w8)
            whi3 = p_sm.tile([128, NH, NIT], dt.float8e4, tag="whi3",
                             name="whi3")
            wlo3 = p_sm.tile([128, NH, NIT], dt.float8e4, tag="wlo3",
                             name="wlo3")
            for h in range(NH):
                wsw = p_sm.tile([128, NIT], dt.float32, tag="wsw", name="wsw",
                                bufs=2)
                nc.vector.tensor_scalar_mul(wsw[:], w_sb[h][:], SW)
                nc.vector.tensor_copy(whi3[:, h, :], wsw[:])
                wr = p_sm.tile([128, NIT], dt.float32, tag="wr", name="wr",
                               bufs=2)
                nc.vector.tensor_tensor(wr[:], wsw[:], whi3[:, h, :],
                                        ALU.subtract)
                nc.vector.tensor_scalar_mul(wlo3[:, h, :], wr[:], 16.0)
            for it in range(NIT):
                rows = slice(it * 128, (it + 1) * 128)
                wt = p_sm.tile([128, WCOLS], dt.float8e4, tag="wt", name="wt",
                               bufs=2)
                nc.vector.memset(wt[:], 0.0)
                nc.vector.tensor_copy(wt[:, 0:NH], whi3[:, :, it])
                nc.vector.tensor_copy(wt[:, NH:2 * NH], wlo3[:, :, it])
                nc.gpsimd.dma_start(gsA[rows, 0:WCOLS], wt[:])

            # h = x@W, scaled to G as each tile drains; gather A as soon as
            # head 0 is done, so the denominator and head-0 tiles unblock.
            with tc.tile_pool(name="psA", bufs=2, space="PSUM") as ps_a:
                for h in range(NH):
                    for it in range(NIT):
                        ps = ps_a.tile([128, HID], dt.float32, tag="psA", name="psA")
                        for ft in range(NFT):
                            xh = xhi(ft, it * 128, (it + 1) * 128)
                            nc.tensor.matmul(ps[:, 0:512], xh, wsl(h, ft, 0, 512),
                                             start=(ft == 0), stop=(ft == NFT - 1))
                            nc.tensor.matmul(ps[:, 512:HID], xh,
                                             wsl(h, ft, 512, HID),
                                             start=(ft == 0), stop=(ft == NFT - 1))
                        g = p_gt.tile([128, HID], dt.float8e4, tag="g0",
                                      name="g0", bufs=4)
                        nc.vector.tensor_scalar_mul(g[:], ps[:],
                                                    w8_sb[h][:, it:it + 1])
                        rows = slice(it * 128, (it + 1) * 128)
                        if h == 0:
                            nc.sync.dma_start(gsA[rows, WCOLS:GA], g[:])
                        else:
                            eng = nc.sync if h == 1 else nc.scalar
                            eng.dma_start(gsB[rows, (h - 1) * HID:h * HID], g[:])
                    if h == 0:
                        nc.gpsimd.collective_compute(
                            "AllGather", ALU.bypass, replica_groups=rg,
                            ins=[gsA[:]], outs=[gfA[:]])
                nc.gpsimd.collective_compute(
                    "AllGather", ALU.bypass, replica_groups=rg,
                    ins=[gsB[:]], outs=[gfB[:]])
            # bf16 adjacency for the L2 lhsT — queue after the G staging
            for half in range(2):
                t = p_adjt.tile([128, NJT // 2, SLAB], dt.bfloat16, tag="adjt",
                                name="adjt", bufs=2)
                eng = nc.sync if half == 0 else nc.scalar
                eng.dma_start(t[:], adjT_t[half])
                adjt_all.append(t)

        # ---------------- L1 adjacency matmul + epilogue + layer 2 ----------
        with tc.tile_pool(name="xct", bufs=1) as p_xct:
            with (
                tc.tile_pool(name="gst", bufs=8) as p_gst,
                tc.tile_pool(name="etmp", bufs=1) as p_et,
                tc.tile_pool(name="wo", bufs=1) as p_wo,
                tc.tile_pool(name="l2a", bufs=1) as p_l2a,
                tc.tile_pool(name="ps1", bufs=4, space="PSUM") as ps_1,
                tc.tile_pool(name="psh2", bufs=4, space="PSUM") as ps_h2,
            ):
                # Wo loads early; they only feed the inline h2 matmuls
                wo_sb = []
                Wo_t = Wo_d.rearrange("(ot p) c -> ot p c", p=128)
                for ot in range(NCT):
                    t = p_wo.tile([128, G2C], dt.bfloat16, tag="wo", name="wo",
                                  bufs=NCT)
                    eng = nc.sync if ot % 2 == 0 else nc.scalar
                    eng.dma_start(t[:], Wo_t[ot])
                    wo_sb.append(t)

                # feature col-tiles, head-major; epilogue + h2 inline per ct.
                # DoubleRow fp8: each matmul contracts a j-PAIR (256 nodes).
                # ct 0 also carries the w columns (first WCOLS of gfA), so the
                # denominator matmuls ride its tile loads — no separate
                # strided gather of w.
                gvA = gfA.rearrange("(jb jj i p) c -> jb p jj i c",
                                    jj=2, i=2, p=128)
                gvB = gfB.rearrange("(jb jj i p) c -> jb p jj i c",
                                    jj=2, i=2, p=128)
                ps2l = [ps_h2.tile([128, G2C], dt.float32, tag="psh2",
                                   name="psh2") for _ in range(NIT)]
                rbc = [None] * NH
                psd = ps_1.tile([NH, 2, SLAB], dt.float32, tag="psd",
                                name="psd", bufs=1)
                DR = mybir.MatmulPerfMode.DoubleRow
                for ct in range(NCT):
                    h = ct // NFT
                    lct = ct % NFT
                    ps = ps_1.tile([128, SLAB], dt.float32, tag="ps1", name="ps1",
                                   bufs=2)
                    for jb in range(NJJ // 2):
                        if ct == 0:
                            gt = p_gst.tile([128, 2, 2, WCOLS + 128],
                                            dt.float8e4, tag="gst0",
                                            name="gst0", bufs=8)
                            eng = nc.sync if jb % 2 == 0 else nc.scalar
                            eng.dma_start(gt[:], gvA[jb, :, :, :, 0:WCOLS + 128])
                            goff = WCOLS
                        elif h == 0:
                            gt = p_gst.tile([128, 2, 2, 128], dt.float8e4,
                                            tag="gst", name="gst")
                            eng = nc.sync if jb % 2 == 0 else nc.scalar
                            eng.dma_start(gt[:], gvA[jb, :, :, :,
                                                     WCOLS + lct * 128:
                                                     WCOLS + (lct + 1) * 128])
                            goff = 0
                        else:
                            gt = p_gst.tile([128, 2, 2, 128], dt.float8e4,
                                            tag="gst", name="gst")
                            eng = nc.sync if jb % 2 == 0 else nc.scalar
                            c0 = (h - 1) * HID + lct * 128
                            eng.dma_start(gt[:], gvB[jb, :, :, :, c0:c0 + 128])
                            goff = 0
                        for q in range(2):
                            jj = jb * 2 + q
                            if ct == 0:
                                nc.tensor.matmul(psd[:, 0, :],
                                                 gt[:, q, :, 0:NH],
                                                 adjd(jj), start=(jj == 0),
                                                 stop=(jj == NJJ - 1),
                                                 perf_mode=DR)
                                nc.tensor.matmul(psd[:, 1, :],
                                                 gt[:, q, :, NH:2 * NH],
                                                 adjd(jj), start=(jj == 0),
                                                 stop=(jj == NJJ - 1),
                                                 perf_mode=DR)
                            nc.tensor.matmul(ps[:],
                                             gt[:, q, :, goff:goff + 128],
                                             adjd(jj), start=(jj == 0),
                                             stop=(jj == NJJ - 1),
                                             perf_mode=DR)
                    if ct == 0:
                        # den = (psd_hi + psd_lo/16); recip carries SW/SG
                        dlo = p_et.tile([NH, SLAB], dt.float32, tag="dlo",
                                        name="dlo")
                        nc.vector.tensor_copy(dlo[:], psd[:, 1, :])
                        den3 = p_et.tile([NH, SLAB], dt.float32, tag="den3",
                                         name="den3")
                        nc.vector.scalar_tensor_tensor(den3[:], dlo[:],
                                                       1.0 / 16.0, psd[:, 0, :],
                                                       ALU.mult, ALU.add)
                        recip3 = p_et.tile([NH, SLAB], dt.float32, tag="recip3",
                                           name="recip3")
                        nc.vector.reciprocal(recip3[:], den3[:])
                        nc.vector.tensor_scalar_mul(recip3[:], recip3[:],
                                                    SW / SG)
                        for hh in range(NH):
                            rrow = p_et.tile([1, SLAB], dt.float32, tag="rrow",
                                             name="rrow", bufs=2)
                            nc.sync.dma_start(rrow[:], recip3[hh:hh + 1, :])
                            rb = p_et.tile([128, SLAB], dt.float32, tag="rbc",
                                           name="rbc", bufs=NH)
                            nc.gpsimd.partition_broadcast(rb[:], rrow[:],
                                                          channels=128)
                            rbc[hh] = rb
                    # xcatT tile = elu(numT / den), bf16
                    z = p_et.tile([128, SLAB], dt.float32, tag="z", name="z",
                                  bufs=2)
                    nc.vector.tensor_tensor(z[:], ps[:], rbc[h][:], ALU.mult)
                    e = p_et.tile([128, SLAB], dt.float32, tag="e", name="e",
                                  bufs=2)
                    nc.scalar.activation(e[:], z[:], AF.Exp)
                    nc.vector.tensor_scalar(e[:], e[:], 1.0, -1.0, ALU.min,
                                            ALU.add)
                    xc = p_xct.tile([128, SLAB], dt.bfloat16, tag="xcp",
                                    name="xcp", bufs=NCT)
                    nc.vector.scalar_tensor_tensor(xc[:], z[:], 0.0, e[:],
                                                   ALU.max, ALU.add)
                    # layer 2 accumulation: h2 += xcat_ct @ Wo_ct
                    for it in range(NIT):
                        nc.tensor.matmul(ps2l[it][:],
                                         xc[:, it * 128:(it + 1) * 128],
                                         wo_sb[ct][:],
                                         start=(ct == 0), stop=(ct == NCT - 1))

                # layer-2 weights w2 = exp(s2') with NO max subtraction:
                # s2' stays well under fp32/bf16 exp range and the common
                # scale cancels exactly in num/den.
                for it in range(NIT):
                    rows = slice((it % 2) * 128, (it % 2 + 1) * 128)
                    w2 = p_l2a.tile([128, 1], dt.float32, tag="w2", name="w2",
                                    bufs=2)
                    nc.scalar.activation(w2[:], ps2l[it][:, NCLS:G2C], AF.Exp)
                    g2b = p_l2a.tile([128, PAD2], dt.bfloat16, tag="g2b",
                                     name="g2b", bufs=2)
                    nc.vector.tensor_scalar_mul(g2b[:, 0:NCLS],
                                                ps2l[it][:, 0:NCLS], w2[:])
                    nc.vector.tensor_copy(g2b[:, NCLS:G2C], w2[:])
                    nc.vector.memset(g2b[:, G2C:PAD2], 0.0)
                    nc.sync.dma_start(g2_slab[it // 2][rows, :], g2b[:])
                    if it == 1:
                        nc.gpsimd.collective_compute(
                            "AllGather", ALU.bypass, replica_groups=rg,
                            ins=[g2_slab[0][:]], outs=[g2_full[0][:]])
                nc.gpsimd.collective_compute(
                    "AllGather", ALU.bypass, replica_groups=rg,
                    ins=[g2_slab[1][:]], outs=[g2_full[1][:]])

            # L2 adjacency matmul + final epilogue
            with (
                tc.tile_pool(name="g2t", bufs=NJT) as p_g2t,
                tc.tile_pool(name="fin", bufs=1) as p_f,
                tc.tile_pool(name="ps2", bufs=4, space="PSUM") as ps_2,
            ):
                g2tiles = [[], []]
                for k in range(2):
                    g2v = g2_full[k].rearrange("(t p) c -> t p c", p=128)
                    for t in range(NJT // 2):
                        gt2 = p_g2t.tile([128, PAD2], dt.bfloat16, tag="g2t",
                                         name="g2t")
                        eng = nc.sync if t % 2 == 0 else nc.scalar
                        eng.dma_start(gt2[:], g2v[t])
                        g2tiles[k].append(gt2)
                ps2 = [ps_2.tile([128, G2C], dt.float32, tag="ps2", name="ps2")
                       for _ in range(NIT)]
                for k in range(2):
                    for it in range(NIT):
                        for t in range(NJT // 2):
                            jt = (t // 2) * 4 + k * 2 + (t % 2)
                            lhs = adjs(jt, it * 128, (it + 1) * 128)
                            nc.tensor.matmul(ps2[it][:],
                                             lhs, g2tiles[k][t][:, 0:G2C],
                                             start=(k == 0 and t == 0),
                                             stop=(k == 1 and t == NJT // 2 - 1))
                for it in range(NIT):
                    r2 = p_f.tile([128, 1], dt.float32, tag="r2", name="r2",
                                  bufs=2)
                    nc.vector.reciprocal(r2[:], ps2[it][:, NCLS:G2C])
                    z = p_f.tile([128, NCLS], dt.float32, tag="z2", name="z2",
                                 bufs=2)
                    nc.vector.tensor_scalar_mul(z[:], ps2[it][:, 0:NCLS], r2[:])
                    e = p_f.tile([128, NCLS], dt.float32, tag="e2", name="e2",
                                 bufs=2)
                    nc.scalar.activation(e[:], z[:], AF.Exp)
                    nc.vector.tensor_scalar(e[:], e[:], 1.0, -1.0, ALU.min,
                                            ALU.add)
                    o = p_f.tile([128, NCLS], dt.float32, tag="o2", name="o2",
                                 bufs=2)
                    nc.vector.scalar_tensor_tensor(o[:], z[:], 0.0, e[:],
                                                   ALU.max, ALU.add)
                    # log_softmax without max subtraction: o <= ~10, so
                    # exp stays comfortably inside fp32 range
                    t = p_f.tile([128, NCLS], dt.float32, tag="texp", name="texp",
                                 bufs=2)
                    nc.scalar.activation(t[:], o[:], AF.Exp)
                    ssum = p_f.tile([128, 1], dt.float32, tag="ssum", name="ssum",
                                    bufs=2)
                    nc.vector.tensor_reduce(ssum[:], t[:],
                                            axis=mybir.AxisListType.X,
                                            op=ALU.add)
                    lg = p_f.tile([128, 1], dt.float32, tag="lg", name="lg",
                                  bufs=2)
                    nc.scalar.activation(lg[:], ssum[:], AF.Ln)
                    fin = p_f.tile([128, NCLS], dt.float32, tag="fin", name="fin",
                                   bufs=2)
                    nc.vector.tensor_scalar(fin[:], o[:], lg[:], None,
                                            ALU.subtract)
                    nc.sync.dma_start(out_d[it * 128:(it + 1) * 128, :], fin[:])

    nc.finalize()
    return nc


_CACHE = {}


def _pair(a):
    hi = a.astype(BF16)
    lo = (a - hi.astype(np.float32)).astype(BF16)
    return hi, lo


def prepare_inputs(x, adj, W_heads, a_heads, W_out, a_out):
    """Shard + lay out the full inputs for the 8 cores."""
    x2 = np.asarray(x, np.float32)[0]          # [N, F]
    adj2 = np.asarray(adj)[0]                  # [N, N] int32
    W3 = np.asarray(W_heads, np.float32).reshape(NH, F, HID)
    a3 = np.asarray(a_heads, np.float32)       # [NH, 2*HID, 1]
    Wo = np.asarray(W_out, np.float32).reshape(GH_TOT, NCLS)
    ao = np.asarray(a_out, np.float32)         # [2*NCLS, 1]

    # fold the edge-score projections into the weights:
    #   s2 = x @ (W @ a2),   s2' = xcat @ (Wo @ ao2)
    u = np.einsum("hfo,ho->hf", W3.astype(np.float64),
                  a3[:, HID:, 0].astype(np.float64)).astype(np.float32)  # [NH,F]
    u_hi, u_lo = _pair(u)
    U6 = np.zeros((F, 8), BF16)
    for h in range(NH):
        U6[:, 2 * h] = u_hi[h]
        U6[:, 2 * h + 1] = u_lo[h]
    u2 = (Wo.astype(np.float64) @ ao[NCLS:, 0].astype(np.float64)).astype(np.float32)
    Wo_ext = np.concatenate([Wo, u2[:, None]], axis=1)       # [GH, 257]
    Wo_b = Wo_ext.astype(BF16)
    W_b = W3.astype(BF16)
    xT = np.ascontiguousarray(x2.T)            # [F, N]
    adjb = adj2.astype(BF16)                   # exact 0/1

    # exact per-head max of s2 = x @ u, folded on the host so the device
    # needs no max-reduction collective.  Mirror the device arithmetic
    # (bf16 x_hi against the u hi/lo pair, accumulated in fp32).
    xh_f = x2.astype(BF16).astype(np.float32)
    s2 = (xh_f @ u_hi.T.astype(np.float32)
          + xh_f @ u_lo.T.astype(np.float32))                     # [N, NH]
    negC = -s2.max(axis=0, keepdims=True).astype(np.float32)      # [1, NH]

    in_maps = []
    for c in range(NCORES):
        sl = slice(c * SLAB, (c + 1) * SLAB)
        xh = np.ascontiguousarray(xT[:, sl]).astype(BF16)
        adjTc = np.ascontiguousarray(adjb[sl, :].T)
        in_maps.append({
            "adjT": adjTc,
            "adjT8": adjTc.astype(F8E4),
            "xT_hi": xh,
            "U6": U6, "negC": negC,
            "W": W_b, "Wo": Wo_b,
        })
    return in_maps


def kernel(x, adj, W_heads, a_heads, W_out, a_out):
    if "nc" not in _CACHE:
        # touch the devices once so any residual bad state from a previous
        # process surfaces (and clears) before the real run
        try:
            import jax
            jax.block_until_ready(jax.numpy.zeros(8))
        except Exception:
            pass
        _CACHE["nc"] = build()
    nc = _CACHE["nc"]
    in_maps = prepare_inputs(x, adj, W_heads, a_heads, W_out, a_out)
    res = run_bass_kernel_spmd(nc, in_maps, list(range(NCORES)))
    out = np.concatenate([res.results[c]["out"] for c in range(NCORES)], axis=0)
    return out.reshape(1, N, NCLS)


# revision 56
# speedup vs baseline: 2.5754x; 1.0558x over previous
"""GAT (2-layer, 3-head) forward on 8 Trainium2 NeuronCores.

Math: with LeakyReLU slope ALPHA=1.0 the edge score e_ij = s1_i + s2_j is
linear, and s1_i cancels inside the row softmax.  The masked softmax over
j therefore reduces to column weights w_j = exp(s2_j - C) restricted to
adj, giving

    h'_i = (sum_j adj_ij * w_j * h_j) / (sum_j adj_ij * w_j)

i.e. one adjacency matmul against G = [w*h | w].  Both GAT layers take
this form (the same adjacency masks both), so the whole network is two
A-matmuls plus small projections.

Sharding: rows of h' (nodes) across 8 cores; each core holds lhsT-layout
adjacency columns A^T[:, slab] and computes its 512-row slab.  Matmuls
run in single bf16 (the tolerance is 2e-2; only the edge-score s2, which
sits in an exponent, is kept in ~fp32 via a hi/lo pair trick folded into
the input prep).  G is gathered in two chunks: [G0 | w] first so the
denominator and head-0 tiles unblock early, then [G1 | G2].
"""
import sys

sys.path.insert(0, "/opt/trn_rl_repo")

import numpy as np
import ml_dtypes

import concourse.bass as bass
import concourse.bacc as bacc
import concourse.mybir as mybir
import concourse.bass_isa as bass_isa
import concourse.tile as tile
from concourse.bass_utils import run_bass_kernel_spmd

BF16 = ml_dtypes.bfloat16
F8E4 = ml_dtypes.float8_e4m3

N = 4096
F = 768
HID = 768
NH = 3
NCLS = 256
NCORES = 8
SLAB = N // NCORES          # 512 rows per core
NIT = SLAB // 128           # 4 i-tiles per core
NJT = N // 128              # 32 j-tiles
NFT = F // 128              # 6 f-tiles
NCT = NH * NFT              # 18 feature col-tiles of G
G2C = NCLS + 1              # 257 = classes + s2' column (folded u2)
PAD2 = 264                  # G2 padded to 32B rows
WCOLS = 32                  # w-column slab width (6 used + pad, 32B rows)
GA = WCOLS + HID            # gather-A width: [w cols | head0 G]
GB = 2 * HID                # gather-B width: head1 + head2 G
GH_TOT = NH * HID           # 2304 xcat feature rows of Wo
SG = 8.0                    # fp8 scale on G ( |G*8| << 240 )
SW = 128.0                  # fp8 scale on w (w <= 1)
NJJ = NJT // 2              # 16 j-pair blocks for DoubleRow

AF = mybir.ActivationFunctionType
ALU = mybir.AluOpType


def _enable_ldw_opt():
    # walrus defaults to --enable-ldw-opt=false; with it off every LDWEIGHTS
    # serializes against the previous matmul (~427ns vs ~213ns per 512-col
    # matmul).  Patch the arg builder so the stationary loads pipeline.
    import concourse.bass_utils as _bu
    if getattr(_bu, "_ldw_opt_patched", False):
        return
    _orig = _bu.get_walrus_args

    def _patched(*a, **k):
        args = _orig(*a, **k)
        return [x.replace("--enable-ldw-opt=false", "--enable-ldw-opt=true")
                for x in args]

    _bu.get_walrus_args = _patched
    _bu._ldw_opt_patched = True


def build():
    dt = mybir.dt
    _enable_ldw_opt()
    nc = bacc.Bacc(num_devices=NCORES)

    adjT8_d = nc.dram_tensor("adjT8", [N, SLAB], dt.float8e4, kind="ExternalInput")
    adjT_d = nc.dram_tensor("adjT", [N, SLAB], dt.bfloat16, kind="ExternalInput")
    xTh_d = nc.dram_tensor("xT_hi", [F, SLAB], dt.bfloat16, kind="ExternalInput")
    U6_d = nc.dram_tensor("U6", [F, 8], dt.bfloat16, kind="ExternalInput")
    # negC[0, h] = -max_i s2_i(head h), computed exactly on the host
    negC_d = nc.dram_tensor("negC", [1, NH], dt.float32, kind="ExternalInput")
    W_d = nc.dram_tensor("W", [NH, F, HID], dt.bfloat16, kind="ExternalInput")
    Wo_d = nc.dram_tensor("Wo", [GH_TOT, G2C], dt.bfloat16, kind="ExternalInput")
    out_d = nc.dram_tensor("out", [SLAB, NCLS], dt.float32, kind="ExternalOutput")

    # DRAM scratch + collective buffers (fp8: halves gather + reload bytes)
    gsA = nc.dram_tensor("gsA", [SLAB, GA], dt.float8e4)
    gfA = nc.dram_tensor("gfA", [N, GA], dt.float8e4, addr_space="Shared")
    gsB = nc.dram_tensor("gsB", [SLAB, GB], dt.float8e4)
    gfB = nc.dram_tensor("gfB", [N, GB], dt.float8e4, addr_space="Shared")
    # g2 gathered in two half-slab chunks so the L2 matmul can start on the
    # first half while the second is in flight
    g2_slab = [nc.dram_tensor(f"g2_slab{k}", [SLAB // 2, PAD2], dt.bfloat16)
               for k in range(2)]
    g2_full = [nc.dram_tensor(f"g2_full{k}", [N // 2, PAD2], dt.bfloat16,
                              addr_space="Shared") for k in range(2)]

    rg = [list(range(NCORES))]

    with tile.TileContext(nc) as tc:
      with tc.tile_pool(name="adjt", bufs=NJT) as p_adjt:
        # ---------------- phase 1: s2, w, h=x@W, G build + gathers ----------
        with (
            tc.tile_pool(name="xw", bufs=1) as p_xw,
            tc.tile_pool(name="small", bufs=1) as p_sm,
            tc.tile_pool(name="gtmp", bufs=1) as p_gt,
        ):
            # Batched input loads: one big DMA per tensor (chunked transfers
            # serialize at ~650ns per 128KB, so 70 small DMAs would cost
            # ~45us of serial load time).  x + head-0 W first: they gate
            # s2 and the first x@W matmuls.
            xh_all = p_xw.tile([128, NFT, SLAB], dt.bfloat16, tag="xh", name="xh")
            nc.sync.dma_start(xh_all[:],
                              xTh_d.rearrange("(ft p) i -> p ft i", p=128))

            def xhi(ft, c0, c1):
                return xh_all[:, ft, c0:c1]

            u6 = p_sm.tile([128, NFT, 8], dt.bfloat16, tag="u6", name="u6")
            nc.gpsimd.dma_start(u6[:], U6_d.rearrange("(ft p) c -> p ft c", p=128))
            negC = p_sm.tile([1, NH], dt.float32, tag="negC", name="negC")
            nc.gpsimd.dma_start(negC[:], negC_d[:])
            negCbc = p_sm.tile([128, NH], dt.float32, tag="negCbc", name="negCbc")
            nc.gpsimd.partition_broadcast(negCbc[:], negC[:], channels=128)

            W_t = W_d.rearrange("h (ft p) o -> p h ft o", p=128)
            w0_all = p_xw.tile([128, NFT, HID], dt.bfloat16, tag="w0", name="w0")
            nc.sync.dma_start(w0_all[:], W_t[:, 0])
            w12_all = p_xw.tile([128, 2, NFT, HID], dt.bfloat16, tag="w12",
                                name="w12")
            nc.scalar.dma_start(w12_all[:], W_t[:, 1:3])

            def wsl(h, ft, c0, c1):
                if h == 0:
                    return w0_all[:, ft, c0:c1]
                return w12_all[:, h - 1, ft, c0:c1]

            # fp8 adjacency, j-pair interleaved for DoubleRow (L1 rhs)
            adj8_all = []
            adjT8_t = adjT8_d.rearrange("(half jj i p) n -> half p jj i n",
                                        half=2, i=2, p=128)
            for half in range(2):
                t = p_adjt.tile([128, NJJ // 2, 2, SLAB], dt.float8e4,
                                tag="adj8", name="adj8", bufs=2)
                eng = nc.sync if half == 0 else nc.scalar
                eng.dma_start(t[:], adjT8_t[half])
                adj8_all.append(t)

            def adjd(jj):
                return adj8_all[jj // (NJJ // 2)][:, jj % (NJJ // 2), :, :]

            # bf16 adjacency per original j-tile (L2 lhsT) — needed only at
            # the tail, loaded after the phase-1 traffic
            adjt_all = []
            adjT_t = adjT_d.rearrange("(half jh p) i -> half p jh i",
                                      half=2, p=128)

            def adjs(j, c0=0, c1=SLAB):
                return adjt_all[j // (NJT // 2)][:, j % (NJT // 2), c0:c1]

            # head-0 x@W FIRST (PE warms up, psum tiles park until w is
            # ready), then the tiny s2 matmuls — its DVE chain and the exp
            # overlap the head-0 compute, so gather A fires ~25us earlier.
            ctx_psA = tc.tile_pool(name="psA", bufs=3, space="PSUM")
            ps_a = ctx_psA.__enter__()

            def xw_head(h, it):
                ps = ps_a.tile([128, HID], dt.float32, tag="psA", name="psA")
                for ft in range(NFT):
                    xh = xhi(ft, it * 128, (it + 1) * 128)
                    nc.tensor.matmul(ps[:, 0:512], xh, wsl(h, ft, 0, 512),
                                     start=(ft == 0), stop=(ft == NFT - 1))
                    nc.tensor.matmul(ps[:, 512:HID], xh, wsl(h, ft, 512, HID),
                                     start=(ft == 0), stop=(ft == NFT - 1))
                return ps

            # head-0 its 0-2 fill the three psum bufs; it3 is emitted after
            # the s2 block so its buffer wait can be satisfied (the it0
            # G-scale frees it once w is ready)
            h0_ps = [xw_head(0, it) for it in range(NIT - 1)]

            # s2 = x_hi @ (u_hi + u_lo): one PSUM bank, no inter-it reuse
            # stalls.  u kept as a bf16 pair; x_hi-only costs ~0.8% on w,
            # which averages out over ~2k neighbours.
            s2_sb = []
            for h in range(NH):
                s2_sb.append(p_sm.tile([128, NIT], dt.float32, tag="s2",
                                       name="s2", bufs=NH))
            with tc.tile_pool(name="psS", bufs=1, space="PSUM") as ps_s:
                p6 = ps_s.tile([128, NIT, 8], dt.float32, tag="p6", name="p6")
                for it in range(NIT):
                    for ft in range(NFT):
                        xh = xhi(ft, it * 128, (it + 1) * 128)
                        nc.tensor.matmul(p6[:, it, :], xh, u6[:, ft, :],
                                         start=(ft == 0), stop=(ft == NFT - 1))
                for it in range(NIT):
                    t6 = p_sm.tile([128, 8], dt.float32, tag="t6", name="t6",
                                   bufs=2)
                    nc.vector.tensor_copy(t6[:], p6[:, it, :])
                    tsum = p_sm.tile([128, NH], dt.float32, tag="tsum",
                                     name="tsum", bufs=2)
                    nc.vector.tensor_tensor(tsum[:], t6[:, 0:2 * NH:2],
                                            t6[:, 1:2 * NH:2], ALU.add)
                    for h in range(NH):
                        nc.vector.tensor_copy(s2_sb[h][:, it:it + 1],
                                              tsum[:, h:h + 1])

            # w = exp(s2 - C) with the host-computed C — no collective needed.
            # Stage w*SW as an fp8 hi/lo pair (hi + lo/16 ≈ 8 mantissa bits)
            # for the DoubleRow denominator matmul, and keep w*SG in fp32 for
            # scaling G.
            w_sb, w8_sb = [], []
            for h in range(NH):
                w = p_sm.tile([128, NIT], dt.float32, tag="wexp", name="wexp",
                              bufs=NH)
                nc.scalar.activation(w[:], s2_sb[h][:], AF.Exp,
                                     bias=negCbc[:, h:h + 1])
                w_sb.append(w)
                w8 = p_sm.tile([128, NIT], dt.float32, tag="wsg", name="wsg",
                               bufs=NH)
                nc.vector.tensor_scalar_mul(w8[:], w[:], SG)
                w8_sb.append(w8)
            whi3 = p_sm.tile([128, NH, NIT], dt.float8e4, tag="whi3",
                             name="whi3")
            wlo3 = p_sm.tile([128, NH, NIT], dt.float8e4, tag="wlo3",
                             name="wlo3")
            for h in range(NH):
                wsw = p_sm.tile([128, NIT], dt.float32, tag="wsw", name="wsw",
                                bufs=2)
                nc.vector.tensor_scalar_mul(wsw[:], w_sb[h][:], SW)
                nc.vector.tensor_copy(whi3[:, h, :], wsw[:])
                wr = p_sm.tile([128, NIT], dt.float32, tag="wr", name="wr",
                               bufs=2)
                nc.vector.tensor_tensor(wr[:], wsw[:], whi3[:, h, :],
                                        ALU.subtract)
                nc.vector.tensor_scalar_mul(wlo3[:, h, :], wr[:], 16.0)
            for it in range(NIT):
                rows = slice(it * 128, (it + 1) * 128)
                wt = p_sm.tile([128, WCOLS], dt.float8e4, tag="wt", name="wt",
                               bufs=2)
                nc.vector.memset(wt[:], 0.0)
                nc.vector.tensor_copy(wt[:, 0:NH], whi3[:, :, it])
                nc.vector.tensor_copy(wt[:, NH:2 * NH], wlo3[:, :, it])
                nc.gpsimd.dma_start(gsA[rows, 0:WCOLS], wt[:])

            # head-0 G build -> gather A fires as early as possible
            for it in range(NIT - 1):
                g = p_gt.tile([128, HID], dt.float8e4, tag="g0",
                              name="g0", bufs=4)
                nc.vector.tensor_scalar_mul(g[:], h0_ps[it][:],
                                            w8_sb[0][:, it:it + 1])
                rows = slice(it * 128, (it + 1) * 128)
                nc.sync.dma_start(gsA[rows, WCOLS:GA], g[:])
            ps = xw_head(0, NIT - 1)
            g = p_gt.tile([128, HID], dt.float8e4, tag="g0", name="g0", bufs=4)
            nc.vector.tensor_scalar_mul(g[:], ps[:],
                                        w8_sb[0][:, NIT - 1:NIT])
            nc.sync.dma_start(gsA[(NIT - 1) * 128:SLAB, WCOLS:GA], g[:])
            nc.gpsimd.collective_compute(
                "AllGather", ALU.bypass, replica_groups=rg,
                ins=[gsA[:]], outs=[gfA[:]])
            h0_ps = None

            # heads 1-2: x@W, scale, stage, gather B
            for h in (1, 2):
                for it in range(NIT):
                    ps = xw_head(h, it)
                    g = p_gt.tile([128, HID], dt.float8e4, tag="g0",
                                  name="g0", bufs=4)
                    nc.vector.tensor_scalar_mul(g[:], ps[:],
                                                w8_sb[h][:, it:it + 1])
                    rows = slice(it * 128, (it + 1) * 128)
                    eng = nc.sync if h == 1 else nc.scalar
                    eng.dma_start(gsB[rows, (h - 1) * HID:h * HID], g[:])
            nc.gpsimd.collective_compute(
                "AllGather", ALU.bypass, replica_groups=rg,
                ins=[gsB[:]], outs=[gfB[:]])
            ctx_psA.__exit__(None, None, None)
            # bf16 adjacency for the L2 lhsT — queue after the G staging
            for half in range(2):
                t = p_adjt.tile([128, NJT // 2, SLAB], dt.bfloat16, tag="adjt",
                                name="adjt", bufs=2)
                eng = nc.sync if half == 0 else nc.scalar
                eng.dma_start(t[:], adjT_t[half])
                adjt_all.append(t)

        # ---------------- L1 adjacency matmul + epilogue + layer 2 ----------
        with tc.tile_pool(name="xct", bufs=1) as p_xct:
            with (
                tc.tile_pool(name="gst", bufs=8) as p_gst,
                tc.tile_pool(name="etmp", bufs=1) as p_et,
                tc.tile_pool(name="wo", bufs=1) as p_wo,
                tc.tile_pool(name="l2a", bufs=1) as p_l2a,
                tc.tile_pool(name="ps1", bufs=4, space="PSUM") as ps_1,
                tc.tile_pool(name="psh2", bufs=4, space="PSUM") as ps_h2,
            ):
                # Wo loads early; they only feed the inline h2 matmuls
                wo_sb = []
                Wo_t = Wo_d.rearrange("(ot p) c -> ot p c", p=128)
                for ot in range(NCT):
                    t = p_wo.tile([128, G2C], dt.bfloat16, tag="wo", name="wo",
                                  bufs=NCT)
                    eng = nc.sync if ot % 2 == 0 else nc.scalar
                    eng.dma_start(t[:], Wo_t[ot])
                    wo_sb.append(t)

                # feature col-tiles, head-major; epilogue + h2 inline per ct.
                # DoubleRow fp8: each matmul contracts a j-PAIR (256 nodes).
                # ct 0 also carries the w columns (first WCOLS of gfA), so the
                # denominator matmuls ride its tile loads — no separate
                # strided gather of w.
                gvA = gfA.rearrange("(jb jj i p) c -> jb p jj i c",
                                    jj=2, i=2, p=128)
                gvB = gfB.rearrange("(jb jj i p) c -> jb p jj i c",
                                    jj=2, i=2, p=128)
                ps2l = [ps_h2.tile([128, G2C], dt.float32, tag="psh2",
                                   name="psh2") for _ in range(NIT)]
                rbc = [None] * NH
                psd = ps_1.tile([NH, 2, SLAB], dt.float32, tag="psd",
                                name="psd", bufs=1)
                DR = mybir.MatmulPerfMode.DoubleRow
                for ct in range(NCT):
                    h = ct // NFT
                    lct = ct % NFT
                    ps = ps_1.tile([128, SLAB], dt.float32, tag="ps1", name="ps1",
                                   bufs=2)
                    for jb in range(NJJ // 2):
                        if ct == 0:
                            gt = p_gst.tile([128, 2, 2, WCOLS + 128],
                                            dt.float8e4, tag="gst0",
                                            name="gst0", bufs=8)
                            eng = nc.sync if jb % 2 == 0 else nc.scalar
                            eng.dma_start(gt[:], gvA[jb, :, :, :, 0:WCOLS + 128])
                            goff = WCOLS
                        elif h == 0:
                            gt = p_gst.tile([128, 2, 2, 128], dt.float8e4,
                                            tag="gst", name="gst")
                            eng = nc.sync if jb % 2 == 0 else nc.scalar
                            eng.dma_start(gt[:], gvA[jb, :, :, :,
                                                     WCOLS + lct * 128:
                                                     WCOLS + (lct + 1) * 128])
                            goff = 0
                        else:
                            gt = p_gst.tile([128, 2, 2, 128], dt.float8e4,
                                            tag="gst", name="gst")
                            eng = nc.sync if jb % 2 == 0 else nc.scalar
                            c0 = (h - 1) * HID + lct * 128
                            eng.dma_start(gt[:], gvB[jb, :, :, :, c0:c0 + 128])
                            goff = 0
                        for q in range(2):
                            jj = jb * 2 + q
                            if ct == 0:
                                nc.tensor.matmul(psd[:, 0, :],
                                                 gt[:, q, :, 0:NH],
                                                 adjd(jj), start=(jj == 0),
                                                 stop=(jj == NJJ - 1),
                                                 perf_mode=DR)
                                nc.tensor.matmul(psd[:, 1, :],
                                                 gt[:, q, :, NH:2 * NH],
                                                 adjd(jj), start=(jj == 0),
                                                 stop=(jj == NJJ - 1),
                                                 perf_mode=DR)
                            nc.tensor.matmul(ps[:],
                                             gt[:, q, :, goff:goff + 128],
                                             adjd(jj), start=(jj == 0),
                                             stop=(jj == NJJ - 1),
                                             perf_mode=DR)
                    if ct == 0:
                        # den = (psd_hi + psd_lo/16); recip carries SW/SG
                        dlo = p_et.tile([NH, SLAB], dt.float32, tag="dlo",
                                        name="dlo")
                        nc.vector.tensor_copy(dlo[:], psd[:, 1, :])
                        den3 = p_et.tile([NH, SLAB], dt.float32, tag="den3",
                                         name="den3")
                        nc.vector.scalar_tensor_tensor(den3[:], dlo[:],
                                                       1.0 / 16.0, psd[:, 0, :],
                                                       ALU.mult, ALU.add)
                        recip3 = p_et.tile([NH, SLAB], dt.float32, tag="recip3",
                                           name="recip3")
                        nc.vector.reciprocal(recip3[:], den3[:])
                        nc.vector.tensor_scalar_mul(recip3[:], recip3[:],
                                                    SW / SG)
                        for hh in range(NH):
                            rrow = p_et.tile([1, SLAB], dt.float32, tag="rrow",
                                             name="rrow", bufs=2)
                            nc.sync.dma_start(rrow[:], recip3[hh:hh + 1, :])
                            rb = p_et.tile([128, SLAB], dt.float32, tag="rbc",
                                           name="rbc", bufs=NH)
                            nc.gpsimd.partition_broadcast(rb[:], rrow[:],
                                                          channels=128)
                            rbc[hh] = rb
                    # xcatT tile = elu(numT / den), bf16
                    z = p_et.tile([128, SLAB], dt.float32, tag="z", name="z",
                                  bufs=2)
                    nc.vector.tensor_tensor(z[:], ps[:], rbc[h][:], ALU.mult)
                    e = p_et.tile([128, SLAB], dt.float32, tag="e", name="e",
                                  bufs=2)
                    nc.scalar.activation(e[:], z[:], AF.Exp)
                    nc.vector.tensor_scalar(e[:], e[:], 1.0, -1.0, ALU.min,
                                            ALU.add)
                    xc = p_xct.tile([128, SLAB], dt.bfloat16, tag="xcp",
                                    name="xcp", bufs=NCT)
                    nc.vector.scalar_tensor_tensor(xc[:], z[:], 0.0, e[:],
                                                   ALU.max, ALU.add)
                    # layer 2 accumulation: h2 += xcat_ct @ Wo_ct
                    for it in range(NIT):
                        nc.tensor.matmul(ps2l[it][:],
                                         xc[:, it * 128:(it + 1) * 128],
                                         wo_sb[ct][:],
                                         start=(ct == 0), stop=(ct == NCT - 1))

                # layer-2 weights w2 = exp(s2') with NO max subtraction:
                # s2' stays well under fp32/bf16 exp range and the common
                # scale cancels exactly in num/den.
                for it in range(NIT):
                    rows = slice((it % 2) * 128, (it % 2 + 1) * 128)
                    w2 = p_l2a.tile([128, 1], dt.float32, tag="w2", name="w2",
                                    bufs=2)
                    nc.scalar.activation(w2[:], ps2l[it][:, NCLS:G2C], AF.Exp)
                    g2b = p_l2a.tile([128, PAD2], dt.bfloat16, tag="g2b",
                                     name="g2b", bufs=2)
                    nc.vector.tensor_scalar_mul(g2b[:, 0:NCLS],
                                                ps2l[it][:, 0:NCLS], w2[:])
                    nc.vector.tensor_copy(g2b[:, NCLS:G2C], w2[:])
                    nc.vector.memset(g2b[:, G2C:PAD2], 0.0)
                    nc.sync.dma_start(g2_slab[it // 2][rows, :], g2b[:])
                    if it == 1:
                        nc.gpsimd.collective_compute(
                            "AllGather", ALU.bypass, replica_groups=rg,
                            ins=[g2_slab[0][:]], outs=[g2_full[0][:]])
                nc.gpsimd.collective_compute(
                    "AllGather", ALU.bypass, replica_groups=rg,
                    ins=[g2_slab[1][:]], outs=[g2_full[1][:]])

            # L2 adjacency matmul + final epilogue
            with (
                tc.tile_pool(name="g2t", bufs=NJT) as p_g2t,
                tc.tile_pool(name="fin", bufs=1) as p_f,
                tc.tile_pool(name="ps2", bufs=4, space="PSUM") as ps_2,
            ):
                g2tiles = [[], []]
                for k in range(2):
                    g2v = g2_full[k].rearrange("(t p) c -> t p c", p=128)
                    for t in range(NJT // 2):
                        gt2 = p_g2t.tile([128, PAD2], dt.bfloat16, tag="g2t",
                                         name="g2t")
                        eng = nc.sync if t % 2 == 0 else nc.scalar
                        eng.dma_start(gt2[:], g2v[t])
                        g2tiles[k].append(gt2)
                # one psum tile, 512-col (bank-aligned) stride per it
                ps2 = ps_2.tile([128, NIT, 512], dt.float32, tag="ps2",
                                name="ps2", bufs=1)
                for k in range(2):
                    for it in range(NIT):
                        for t in range(NJT // 2):
                            jt = (t // 2) * 4 + k * 2 + (t % 2)
                            lhs = adjs(jt, it * 128, (it + 1) * 128)
                            nc.tensor.matmul(ps2[:, it, 0:G2C],
                                             lhs, g2tiles[k][t][:, 0:G2C],
                                             start=(k == 0 and t == 0),
                                             stop=(k == 1 and t == NJT // 2 - 1))
                # batched final epilogue: elu + log_softmax on all 4 i-tiles
                # at once (o <= ~10, so exp needs no max subtraction)
                z4 = p_f.tile([128, NIT, NCLS], dt.float32, tag="z4", name="z4")
                for it in range(NIT):
                    r2 = p_f.tile([128, 1], dt.float32, tag="r2", name="r2",
                                  bufs=2)
                    nc.vector.reciprocal(r2[:], ps2[:, it, NCLS:G2C])
                    nc.vector.tensor_scalar_mul(z4[:, it, :],
                                                ps2[:, it, 0:NCLS], r2[:])
                e4 = p_f.tile([128, NIT, NCLS], dt.float32, tag="e4", name="e4")
                nc.scalar.activation(e4[:], z4[:], AF.Exp)
                nc.vector.tensor_scalar(e4[:], e4[:], 1.0, -1.0, ALU.min,
                                        ALU.add)
                o4 = p_f.tile([128, NIT, NCLS], dt.float32, tag="o4", name="o4")
                nc.vector.scalar_tensor_tensor(o4[:], z4[:], 0.0, e4[:],
                                               ALU.max, ALU.add)
                t4 = p_f.tile([128, NIT, NCLS], dt.float32, tag="t4", name="t4")
                nc.scalar.activation(t4[:], o4[:], AF.Exp)
                ssum4 = p_f.tile([128, NIT, 1], dt.float32, tag="ssum4",
                                 name="ssum4")
                nc.vector.tensor_reduce(ssum4[:], t4[:],
                                        axis=mybir.AxisListType.X, op=ALU.add)
                lg4 = p_f.tile([128, NIT, 1], dt.float32, tag="lg4", name="lg4")
                nc.scalar.activation(lg4[:], ssum4[:], AF.Ln)
                fin4 = p_f.tile([128, NIT, NCLS], dt.float32, tag="fin4",
                                name="fin4")
                for it in range(NIT):
                    nc.vector.tensor_scalar(fin4[:, it, :], o4[:, it, :],
                                            lg4[:, it, :], None, ALU.subtract)
                nc.sync.dma_start(
                    out_d.rearrange("(it p) c -> p it c", p=128), fin4[:])

    nc.finalize()
    return nc


_CACHE = {}


def _pair(a):
    hi = a.astype(BF16)
    lo = (a - hi.astype(np.float32)).astype(BF16)
    return hi, lo


def prepare_inputs(x, adj, W_heads, a_heads, W_out, a_out):
    """Shard + lay out the full inputs for the 8 cores."""
    x2 = np.asarray(x, np.float32)[0]          # [N, F]
    adj2 = np.asarray(adj)[0]                  # [N, N] int32
    W3 = np.asarray(W_heads, np.float32).reshape(NH, F, HID)
    a3 = np.asarray(a_heads, np.float32)       # [NH, 2*HID, 1]
    Wo = np.asarray(W_out, np.float32).reshape(GH_TOT, NCLS)
    ao = np.asarray(a_out, np.float32)         # [2*NCLS, 1]

    # fold the edge-score projections into the weights:
    #   s2 = x @ (W @ a2),   s2' = xcat @ (Wo @ ao2)
    u = np.einsum("hfo,ho->hf", W3.astype(np.float64),
                  a3[:, HID:, 0].astype(np.float64)).astype(np.float32)  # [NH,F]
    u_hi, u_lo = _pair(u)
    U6 = np.zeros((F, 8), BF16)
    for h in range(NH):
        U6[:, 2 * h] = u_hi[h]
        U6[:, 2 * h + 1] = u_lo[h]
    u2 = (Wo.astype(np.float64) @ ao[NCLS:, 0].astype(np.float64)).astype(np.float32)
    Wo_ext = np.concatenate([Wo, u2[:, None]], axis=1)       # [GH, 257]
    Wo_b = Wo_ext.astype(BF16)
    W_b = W3.astype(BF16)
    xT = np.ascontiguousarray(x2.T)            # [F, N]
    adjb = adj2.astype(BF16)                   # exact 0/1

    # exact per-head max of s2 = x @ u, folded on the host so the device
    # needs no max-reduction collective.  Mirror the device arithmetic
    # (bf16 x_hi against the u hi/lo pair, accumulated in fp32).
    xh_f = x2.astype(BF16).astype(np.float32)
    s2 = (xh_f @ u_hi.T.astype(np.float32)
          + xh_f @ u_lo.T.astype(np.float32))                     # [N, NH]
    negC = -s2.max(axis=0, keepdims=True).astype(np.float32)      # [1, NH]

    in_maps = []
    for c in range(NCORES):
        sl = slice(c * SLAB, (c + 1) * SLAB)
        xh = np.ascontiguousarray(xT[:, sl]).astype(BF16)
        adjTc = np.ascontiguousarray(adjb[sl, :].T)
        in_maps.append({
            "adjT": adjTc,
            "adjT8": adjTc.astype(F8E4),
            "xT_hi": xh,
            "U6": U6, "negC": negC,
            "W": W_b, "Wo": Wo_b,
        })
    return in_maps


def kernel(x, adj, W_heads, a_heads, W_out, a_out):
    if "nc" not in _CACHE:
        # touch the devices once so any residual bad state from a previous
        # process surfaces (and clears) before the real run
        try:
            import jax
            jax.block_until_ready(jax.numpy.zeros(8))
        except Exception:
            pass
        _CACHE["nc"] = build()
    nc = _CACHE["nc"]
    in_maps = prepare_inputs(x, adj, W_heads, a_heads, W_out, a_out)
    res = run_bass_kernel_spmd(nc, in_maps, list(range(NCORES)))
    out = np.concatenate([res.results[c]["out"] for c in range(NCORES)], axis=0)
    return out.reshape(1, N, NCLS)
